# revision 35
# baseline (speedup 1.0000x reference)
"""Trainium2 8-core GQA attention kernel (tensor-parallel over heads).

Strategy (8 NeuronCores, SPMD):
  - Core c owns q-heads [4c..4c+4) and kv-head c (GQA groups stay aligned).
  - Phases A (qkv projection + RoPE) and B (attention) are merged per token
    chunk: causality means chunk (b,qc) only attends k-chunks <= qc, so the
    attention for a chunk is emitted right after its projection and the Tile
    scheduler fills attention's exp-latency stalls with projection matmuls.
  - qkvT = wqkv_c^T @ x^T is computed feature-major so Q^T/K^T land in
    [head_dim, tokens] layout; RoPE applied with partition-shifted PSUM
    multiply-adds (bf16 intermediates); V^T -> V via DMA transposes; x^T
    streamed as quarter panels shared by the PSUM m-groups.  Projection runs
    in three m-groups [k,v],[q0,q1],[q2,q3] (chunk 0: two groups of three to
    match the cold DMA ramp): k's rope is emitted first so attention's score
    matmuls never wait on it, and each group's PSUM allocs reuse buffers
    freed a full group earlier (no alloc-wait micro-stalls).
  - Attention scores are computed transposed (S^T[k,q]) so exp(S^T) feeds the
    PV matmul directly (lhsT = V[k,d]) with zero P transposes; fully-masked
    causal blocks are skipped; diagonal blocks restrict score/exp/PV to the
    valid q-range (N = 512-128j) and use a canonical per-j {0,1} mask that
    also zeroes the stale q-prefix; denominators for all 4 heads accumulate
    into one [4, 512] PSUM row-set via indicator-column matmuls, up to 16
    exp-blocks per matmul (summed on DVE), emitted with one-head lag so the
    PE queue never stalls on DVE; 1/den is broadcast across partitions on
    GpSimd (no PE broadcast matmuls); normalization is deferred to the output.
  - A tiny warm-up AllGather issued at t=0 absorbs the one-time collective
    barrier + ncfw cold start (~60us) during the DMA-bound startup; the real
    per-chunk AllGathers then run warm (~25us) and never back-pressure the
    GpSimd queue (whose blocking collective waits would otherwise stall the
    partition_broadcast -> at-multiply -> Vector -> PE chain for ~25us).
  - The AllGather of attention outputs is split into 8 token-chunk
    collectives issued as soon as each chunk's attention completes; phase C
    (out^T = wo_c^T @ attn^T) runs as a solid block at the end.  wo is
    preloaded into SBUF during phase A; the deferred last attention chunk's
    V-transposes ride the then-idle scalar queue; the first NPQ k-tiles of
    phase C's first panel live in a persistent tile prefetched mid-phase-A,
    dodging the SBUF WAR hazard that blocks the pool-allocated panels until
    the last proj chunk retires.
  - Host: inputs are pre-transposed to partition-major DRAM layouts
    ([p, ko, cols] / [p, chunk, ko, t]) so DMA descriptors are multi-KB
    runs; cold start fans the first weight/x loads across the sync, scalar
    AND gpsimd issue queues.
All PE math in bf16 (f32 PSUM accumulation).  HW exec ~900us (from 1151us
baseline): PE is GPIO-power-throttled to ~1.95GHz for ~82% of the span, PE
busy ~93%, remaining idle = cold-start DMA ramp + fixed teardown.
"""

import numpy as np
import ml_dtypes

import concourse.bass as bass
import concourse.mybir as mybir
import concourse.tile as tile
from concourse import bacc
from concourse.bass_utils import run_bass_kernel_spmd

BF16 = mybir.dt.bfloat16
F32 = mybir.dt.float32
HD = 128            # head dim
HHD = HD // 2       # rope half
P = 128             # partitions
QCH = 512           # q-chunk / token-chunk size
KT = 128            # k tile (partition dim)
SCALE = 1.0 / np.sqrt(HD)
DGRP = 16           # exp blocks summed per denominator matmul


def build_graph(NB, S, D, HPC, NCORES):
    """Build the per-core SPMD graph (full-prefill causal schedule)."""
    TOK = NB * S
    QF = HPC * HD           # q features per core
    FLOC = QF + 2 * HD      # local qkv features (q + k + v)
    MT = FLOC // P          # feature tiles (q tiles + 1 k + 1 v)
    KD = D // P             # contraction tiles over model dim
    NQC = S // QCH          # q chunks per batch
    NKT = S // KT           # k tiles per batch
    KTC = QCH // KT         # k tiles per token chunk
    ODPC = D // NCORES      # output dims per core
    NCHK = TOK // QCH       # token chunks overall

    nc = bacc.Bacc("TRN2", target_bir_lowering=False, debug=False,
                   num_devices=NCORES)

    # partition-major DRAM layouts (host pre-transposed): per-partition
    # contiguous runs of several KB -> large DMA descriptors, ~1.5-8x the
    # per-engine transfer efficiency of the naive [D, TOK] layout
    xt_d = nc.dram_tensor("xt", [P, NCHK, KD, QCH], BF16,
                          kind="ExternalInput").ap()
    wqkv_d = nc.dram_tensor("wqkv", [P, KD, FLOC], BF16,
                            kind="ExternalInput").ap()
    wo_d = nc.dram_tensor("wo", [P, KD, ODPC], BF16,
                          kind="ExternalInput").ap()
    sc_d = nc.dram_tensor("sincos2", [P, 2 * S], BF16, kind="ExternalInput").ap()
    mask_d = nc.dram_tensor("maskblk", [KTC * P, QCH], BF16,
                            kind="ExternalInput").ap()
    out_d = nc.dram_tensor("out", [ODPC, TOK], BF16,
                           kind="ExternalOutput").ap()

    with tile.TileContext(nc) as tc:
        with tc.tile_pool(name="persist", bufs=1) as persist, \
             tc.tile_pool(name="dram", bufs=1, space="DRAM") as dram:
            kT = persist.tile([P, TOK], BF16)          # K^T, all tokens
            q_def = persist.tile([P, HPC, QCH], BF16)  # deferred chunk's Q^T
            v_kd = persist.tile([P, NB * NKT, HD], BF16)
            wq_sb = persist.tile([P, KD, FLOC], BF16)
            wo_sb = persist.tile([P, KD, ODPC], BF16)
            sc_sb = persist.tile([P, 2 * S], BF16)
            mk = persist.tile([P, KTC, QCH], BF16)     # canonical diag masks
            # indicator columns for per-head denominator batching
            ecol = persist.tile([P, HPC, HPC], BF16)   # [:, h, :] = e_h cols
            nc.vector.memset(ecol[:], 0.0)
            for h in range(HPC):
                nc.vector.memset(ecol[:, h, h:h + 1], 1.0)

            # phase-C chunk 0's first xt-panel quarter, persistent: loaded
            # mid-phase-A (agc[0] is ready ~450us before phase C) so the
            # first wo matmuls never wait on the SBUF WAR hazard that blocks
            # the pool-allocated panel until the last proj chunk retires
            NPQ = 6            # k-tiles of chunk-0's first quarter kept persistent
            agt0q = persist.tile([P, NPQ, QCH], BF16)

            cosT = sc_sb[:, 0:S]
            sinT = sc_sb[:, S:2 * S]

            bounce = [dram.tile([QF, QCH], BF16, name=f"bnc{ci}")
                      for ci in range(NCHK)]
            agc = [dram.tile([QF * NCORES, QCH], BF16, name=f"agc{ci}",
                             addr_space="Shared" if NCORES > 4 else "Local")
                   for ci in range(NCHK)]
            # tiny warm-up AllGather issued first: absorbs the one-time
            # collective barrier + ncfw cold-start (~60us) during the
            # DMA-bound startup, so the first REAL AllGather runs at warm
            # speed (~24us) and never back-pressures the GpSimd queue
            warm_in = dram.tile([1, 64], BF16, name="warm_in")
            warm_out = dram.tile([NCORES, 64], BF16, name="warm_out",
                                 addr_space="Shared" if NCORES > 4 else "Local")

            # ---------- merged phases A (projection+RoPE) and B (attention) --
            with tc.tile_pool(name="phbw", bufs=3) as phbw, \
                 tc.tile_pool(name="psab", bufs=1, space="PSUM") as psab:
              with tc.tile_pool(name="phax", bufs=2) as phax, \
                 tc.tile_pool(name="phaq", bufs=2) as phaq, \
                 tc.tile_pool(name="phat", bufs=2) as phat:

                KQ = KD // 4        # k-tiles per xt quarter

                def load_xt_quarter(col0, quar, tagname, nspl=1):
                    """One quarter of a chunk's x^T panel; the four quarter
                    tiles are shared by all m-groups (no per-group reloads)
                    and rotate a 6-buffer pool so the next chunk's loads
                    overlap this chunk's last-group reads."""
                    xt_sb = phax.tile([P, KQ, QCH], BF16, tag="xt", bufs=5,
                                      name=tagname)
                    ch = col0 // QCH
                    qk = KQ // nspl
                    for q4 in range(nspl):
                        nc.sync.dma_start(
                            xt_sb[:, q4 * qk:(q4 + 1) * qk, :],
                            xt_d[:, ch,
                                 quar * KQ + q4 * qk:
                                 quar * KQ + (q4 + 1) * qk, :])
                    return xt_sb

                # interleave the critical first loads across both queues;
                # after startup the scalar queue carries ONLY exp activations
                # (a 600ns DMA-descriptor issue ahead of an exp delays the
                # whole attention pipeline on the in-order scalar queue)
                # cold start: three issue queues run in parallel.  k0 is
                # split across sync+scalar (first matmul gates on it); wq
                # k1-k7 ride the otherwise-idle gpsimd queue (emitted BEFORE
                # the warm-up AllGather, which then blocks that queue for
                # ~70us); xt quarter-0 subloads alternate sync/scalar in
                # consumption order.
                # cold start: three issue queues in parallel.  k0 is split
                # across sync+scalar (the first matmul gates on it); wq
                # k1-k7 ride the otherwise-idle gpsimd queue (software DMA
                # path, ~18GB/s, emitted BEFORE the warm-up AllGather which
                # then blocks that queue); xt quarter-0 subloads alternate
                # sync/scalar in consumption order.
                HFL = FLOC // 2
                nc.scalar.dma_start(wq_sb[:, 0, 0:HFL], wqkv_d[:, 0, 0:HFL])
                nc.sync.dma_start(wq_sb[:, 0, HFL:FLOC],
                                  wqkv_d[:, 0, HFL:FLOC])
                for ko in range(1, 8):
                    nc.gpsimd.dma_start(wq_sb[:, ko, :], wqkv_d[:, ko, :])
                nc.gpsimd.collective_compute(
                    "AllGather", mybir.AluOpType.bypass,
                    replica_groups=[list(range(NCORES))],
                    ins=[warm_in.opt()], outs=[warm_out.opt()])
                xtf0 = phax.tile([P, KQ, QCH], BF16, tag="xt", bufs=5,
                                 name="xtf0")
                for a, b, eng in ((0, 1, nc.sync), (1, 2, nc.scalar),
                                  (2, 4, nc.sync), (4, 6, nc.scalar),
                                  (6, 8, nc.sync)):
                    eng.dma_start(xtf0[:, a:b, :], xt_d[:, 0, a:b, :])
                xt_q0 = [xtf0]
                # rope tables for position range [0,512) (used by chunk 0 of
                # both batches), then the rest
                nc.scalar.dma_start(sc_sb[:, 0:QCH], sc_d[:, 0:QCH])
                nc.scalar.dma_start(sc_sb[:, S:S + QCH], sc_d[:, S:S + QCH])

                # weight feed split across both queues; early tiles go
                # individually (arrival order matches the first chunk's
                # kg-order consumption), later ones as multi-tile range
                # DMAs whose per-partition-contiguous descriptors run at
                # full engine rate
                def wq_span(k0, k1, step=1):
                    for ko in range(k0, k1, step):
                        hi = min(ko + step, k1)
                        eng = nc.sync if (ko // step) % 2 == 0 else nc.scalar
                        eng.dma_start(
                            wq_sb[:, ko:hi, :], wqkv_d[:, ko:hi, :])

                xt_q0.append(load_xt_quarter(0, 1, "xtf1", nspl=2))
                wq_span(8, 15, step=2)
                xt_q0.append(load_xt_quarter(0, 2, "xtf2", nspl=2))
                wq_span(15, 23, step=4)
                xt_q0.append(load_xt_quarter(0, 3, "xtf3"))
                wq_span(23, KD, step=4)
                # diag masks + rest of rope tables
                nc.sync.dma_start(
                    mk[:], mask_d[:].rearrange("(mb p) q -> p mb q", p=P))
                nc.sync.dma_start(sc_sb[:, QCH:S], sc_d[:, QCH:S])
                nc.sync.dma_start(sc_sb[:, S + QCH:2 * S],
                                  sc_d[:, S + QCH:2 * S])

                # k and v ride in the FIRST group: attention's score matmuls
                # need this chunk's roped kT (emitting k's rope first means it
                # completes ~20us before attention reaches the PE queue,
                # instead of being the very last DVE op).  Three groups of 2
                # also mean each group's PSUM allocs reuse buffers freed
                # early in the group-before-last — no alloc-wait micro-stalls
                # (2 live + 2 next = 4 bufs exactly).  Chunk 0 instead uses
                # two groups of 3: a 2-wide group consumes the xt panel at
                # ~240GB/s, outpacing the cold DMA ramp — 3-wide matches it.
                m_groups = [[MT - 2, MT - 1], [0, 1], [2, 3]]
                m_groups0 = [[MT - 2, MT - 1, 0], [1, 2, 3]]

                def rope_store(pss, dst, s0):
                    """dst(bf16) = pss*cos + rot_half(pss)*sin.

                    The multiplies read PSUM f32 (1 elem/cycle on DVE — a
                    partition-shifted tensor_tensor needs one non-SBUF
                    input anyway); t1/t2 are bf16 so the final add runs at
                    2 elem/cycle.  DVE cost ~2.5us per tile.
                    """
                    t1 = phat.tile([P, QCH], BF16, tag="t1",
                                   name=f"t1_{rope_store.n}")
                    t2 = phat.tile([P, QCH], BF16, tag="t2",
                                   name=f"t2_{rope_store.n}")
                    nc.vector.tensor_mul(t1[:], pss[:], cosT[:, s0:s0 + QCH])
                    nc.vector.tensor_mul(t2[0:HHD, :], pss[HHD:P, :],
                                         sinT[0:HHD, s0:s0 + QCH])
                    nc.vector.tensor_mul(t2[HHD:P, :], pss[0:HHD, :],
                                         sinT[HHD:P, s0:s0 + QCH])
                    nc.vector.tensor_add(dst, t1[:], t2[:])
                    rope_store.n += 1
                rope_store.n = 0

                def proj_filler(b, cb, q_dst):
                    """Chunk (b,cb)'s projection as a filler: fill(n) emits
                    up to n matmuls (returns False when exhausted), with the
                    group-boundary DVE work (ropes / v copy / transposes)
                    emitted as the generator crosses each group.  The
                    previous chunk's attention weaves these matmuls between
                    its exp-paced score/PV pairs, so the PE never drains
                    while the scalar engine catches up."""
                    ch = b * NQC + cb
                    col0 = ch * QCH
                    s0 = col0 % S
                    vT = phaq.tile([P, QCH], BF16, tag="vt", name=f"vT{ch}")
                    if ch == 0:
                        xtq = xt_q0
                    else:
                        xtq = [load_xt_quarter(col0, quar, f"xt{ch}_{quar}")
                               for quar in range(4)]

                    def dve_for(grp, pss):
                        # emit DVE consumers in group-list order (k first)
                        for m in grp:
                            if m == MT - 1:          # v
                                nc.vector.tensor_copy(vT[:], pss[m][:])
                                # V^T -> V transposes: sync queue normally
                                # (a scalar-queue DMA_TRANSPOSE costs 1.2us
                                # of queue occupancy right where attention's
                                # exps live).  The DEFERRED chunk's
                                # attention runs much later (phase C), so
                                # its transposes go on the then-idle scalar
                                # queue instead of behind ~4MB of xt loads
                                # on sync (which starved its PV by ~7us).
                                eng_t = (nc.scalar if ch == NB * NQC - 1
                                         else nc.sync)
                                for j in range(KTC):
                                    kt = cb * KTC + j
                                    eng_t.dma_start_transpose(
                                        v_kd[:, b * NKT + kt, :],
                                        vT[:, j * KT:(j + 1) * KT])
                            elif m == MT - 2:        # k
                                rope_store(pss[m], kT[:, col0:col0 + QCH],
                                           s0)
                            else:                    # q
                                rope_store(pss[m], q_dst[:, m, :], s0)

                    def gen():
                        for grp in (m_groups0 if ch == 0 else m_groups):
                            pss = {m: psab.tile([P, QCH], F32, tag="pa",
                                                bufs=4, name=f"pa{ch}_{m}")
                                   for m in grp}
                            for kg in range(KD):
                                xt_sb = xtq[kg // KQ]
                                for m in grp:
                                    nc.tensor.matmul(
                                        pss[m][:],
                                        wq_sb[:, kg, m * P:(m + 1) * P],
                                        xt_sb[:, kg % KQ, :],
                                        start=(kg == 0),
                                        stop=(kg == KD - 1))
                                    yield True
                            dve_for(grp, pss)

                    g = gen()

                    def fill(n=1):
                        for _ in range(n):
                            if next(g, None) is None:
                                return False
                        return True
                    return fill

                def attn_chunk(ci, b, qc, q_t, full_exp, fill=None):
                    kts = list(range(KTC * (qc + 1)))   # causal prefill
                    # d_ps lives in the "st" pool (scores then run with a
                    # 2-deep lookahead — enough, because fill() weaves proj
                    # matmuls between each score and its exp-gated PV)
                    d_ps = psab.tile([HPC, QCH], F32, tag="st", bufs=3,
                                     name=f"den{ci}")
                    o_tiles = {}
                    n_dgrp = (len(kts) + DGRP - 1) // DGRP
                    dtot = n_dgrp * HPC
                    dcnt = 0
                    step = 0
                    pending = []      # (step_ready, head, dacc)

                    def emit_denoms(lag):
                        nonlocal dcnt
                        while pending and step - pending[0][0] >= lag:
                            _, dh, dacc = pending.pop(0)
                            nc.tensor.matmul(
                                d_ps[:], ecol[:, dh, :], dacc[:],
                                start=(dcnt == 0), stop=(dcnt == dtot - 1))
                            dcnt += 1

                    for h in range(HPC):
                        o_ps = psab.tile([P, QCH], F32, tag="outT", bufs=1,
                                         name=f"o{ci}_{h}")
                        dacc = None
                        dacc_n = 0
                        for i, kt in enumerate(kts):
                            emit_denoms(lag=3)
                            j = kt - KTC * qc
                            qlo = 128 * j if (j >= 1 and not full_exp) else 0
                            st = psab.tile([P, QCH], F32, tag="st", bufs=3,
                                           name=f"st{ci}_{h}_{i}")
                            nc.tensor.matmul(
                                st[:, qlo:QCH],
                                kT[:, b * S + kt * KT:b * S + (kt + 1) * KT],
                                q_t[:, h, qlo:QCH],
                                start=True, stop=True)
                            pt = phbw.tile([P, QCH], BF16, tag="pt", bufs=7,
                                           name=f"pt{ci}_{h}_{i}")
                            nc.scalar.activation(
                                pt[:, qlo:QCH], st[:, qlo:QCH],
                                mybir.ActivationFunctionType.Exp,
                                bias=0.0, scale=float(SCALE))
                            if j >= 0:   # diagonal: mask + zero stale prefix
                                mhi = min(128 * (j + 1), QCH)
                                nc.vector.tensor_mul(pt[:, 0:mhi],
                                                     pt[:, 0:mhi],
                                                     mk[:, j, 0:mhi])
                            if fill is not None:
                                # weave proj matmuls of the NEXT chunk here:
                                # the PE covers this block's exp (+mask)
                                # latency with useful work instead of
                                # idling in-order behind the PV
                                fill(3 if j >= 0 else 2)
                            first, last = (i == 0), (i == len(kts) - 1)
                            nc.tensor.matmul(
                                o_ps[:, qlo:QCH],
                                v_kd[:, b * NKT + kt, :], pt[:, qlo:QCH],
                                start=first, stop=last)
                            # batch up to DGRP exp blocks per denom matmul;
                            # diagonal blocks only contribute on [qlo:] (the
                            # prefix is masked to zero), so restrict the add
                            gpos = i % DGRP
                            if gpos == 0:
                                dacc, dacc_n = pt, 1
                            else:
                                if dacc_n == 1:
                                    dsum = phbw.tile([P, QCH], BF16,
                                                     tag="dsum", bufs=2,
                                                     name=f"ds{ci}_{h}_{i}")
                                    nc.vector.tensor_add(dsum[:], dacc[:],
                                                         pt[:])
                                    dacc = dsum
                                else:
                                    nc.vector.tensor_add(dacc[:, qlo:QCH],
                                                         dacc[:, qlo:QCH],
                                                         pt[:, qlo:QCH])
                                dacc_n += 1
                            if gpos == DGRP - 1 or last:
                                pending.append((step, h, dacc))
                            step += 1
                        o_sb = phbw.tile([P, QCH], BF16, tag="osbuf", bufs=5,
                                         name=f"ou{ci}_{h}")
                        nc.vector.tensor_copy(o_sb[:], o_ps[:])
                        o_tiles[h] = o_sb
                        if fill is not None:
                            fill(2)
                    # flush any remaining denominator matmuls
                    step += 1000
                    emit_denoms(lag=0)
                    assert dcnt == dtot, (dcnt, dtot)
                    inv = phbw.tile([HPC, QCH], F32, tag="inv", bufs=1,
                                    name=f"inv{ci}")
                    nc.vector.reciprocal(inv[:], d_ps[:])
                    invb = phbw.tile([HPC, QCH], BF16, tag="invb", bufs=1,
                                     name=f"invb{ci}")
                    nc.vector.tensor_copy(invb[:], inv[:])
                    # flatten the 4 inv rows onto partition 0 (DMA crosses
                    # partitions freely) so partition_broadcast can expand
                    # them; on the scalar queue, where a short wait on invb
                    # blocks nothing (next exps are ~15us away)
                    invf = phbw.tile([1, HPC * QCH], BF16, tag="invf",
                                     bufs=1, name=f"invf{ci}")
                    nc.scalar.dma_start(invf[:], invb[:])

                    def finalize():
                        # normalization tail; emitted mid-way through the
                        # NEXT projection chunk (before its ropes join the
                        # vector queue).  The inv broadcast runs on GpSimd
                        # (otherwise idle) instead of PE matmuls.
                        for h in range(HPC):
                            bcc = phbw.tile([P, QCH], BF16, tag="bcc",
                                            bufs=1, name=f"bcc{ci}_{h}")
                            nc.gpsimd.partition_broadcast(
                                bcc[:], invf[:, h * QCH:(h + 1) * QCH])
                            at = phbw.tile([P, QCH], BF16, tag="at", bufs=1,
                                           name=f"at{ci}_{h}")
                            nc.vector.tensor_mul(at[:], o_tiles[h][:],
                                                 bcc[:])
                            nc.sync.dma_start(
                                bounce[ci][h * P:(h + 1) * P, :], at[:])
                        nc.gpsimd.collective_compute(
                            "AllGather", mybir.AluOpType.bypass,
                            replica_groups=[list(range(NCORES))],
                            ins=[bounce[ci].opt()], outs=[agc[ci].opt()])
                    return finalize

                # full-prefill causal schedule: attention for chunk qc is
                # WOVEN into the next chunk's projection (its fill matmuls
                # hide the exp chain); the very last attention chunk is
                # deferred past the proj-pool close so phase C's first
                # matmuls can fill its exp-latency stalls
                deferred = (NB - 1, NQC - 1)
                wo_sched = {1: (0, 5), 2: (5, 10), 3: (10, 14),
            4: (14, 19), 5: (19, 23), 6: (23, 28),
            7: (28, KD)}
                chunk_no = 0
                prev = None       # previous chunk's attention, not yet run
                for b in range(NB):
                    for cb in range(NQC):
                        if (b, cb) == deferred:
                            q_dst = q_def
                        else:
                            q_dst = phaq.tile([P, HPC, QCH], BF16, tag="qch",
                                              name=f"q{b}_{cb}")
                        fill = proj_filler(b, cb, q_dst)
                        if prev is not None:
                            fin = attn_chunk(*prev, fill=fill)
                            # normalization + AllGather for the chunk that
                            # just finished its attention
                            fin()
                        while fill(16):
                            pass
                        # trickle wo weight loads behind the xt streams
                        chunk_no += 1
                        if chunk_no in wo_sched:
                            lo, hi = wo_sched[chunk_no]
                            nc.sync.dma_start(wo_sb[:, lo:hi, :],
                                              wo_d[:, lo:hi, :])
                        if chunk_no == 4:
                            # prefetch phase-C chunk 0's first panel quarter
                            # into its persistent home (AG0 finished long
                            # ago; both queues have slack mid-phase)
                            nc.sync.dma_start(
                                agt0q[:],
                                agc[0][0:NPQ * P, :]
                                .rearrange("(ko p) t -> p ko t", p=P))
                        if (b, cb) != deferred:
                            ci = b * NQC + cb
                            prev = (ci, b, cb, q_dst,
                                    b == 0 and cb == 0)
                        else:
                            prev = None

              # -------------- Phase C: out^T = wo_c^T @ attn^T ---------------
              with tc.tile_pool(name="phcx", bufs=2) as phcx, \
                 tc.tile_pool(name="phco", bufs=2) as phco:
                def load_agt(ci, skip_q0=False):
                    agt = phcx.tile([P, KD, QCH], BF16, tag="agt",
                                    name=f"agt{ci}")
                    kq = KD // 4
                    for quar in range(4):
                        # first quarter split across both queues so the
                        # first matmuls start as early as possible
                        if quar == 0:
                            if skip_q0:
                                # k-tiles [0:NPQ) served by the persistent
                                # agt0q; load only the remainder
                                nc.sync.dma_start(
                                    agt[:, NPQ:kq, :],
                                    agc[ci][NPQ * P:kq * P, :]
                                    .rearrange("(ko p) t -> p ko t", p=P))
                                continue
                            h = kq // 2
                            for sub, eng in ((0, nc.sync), (1, nc.scalar)):
                                eng.dma_start(
                                    agt[:, sub * h:(sub + 1) * h, :],
                                    agc[ci][sub * h * P:(sub + 1) * h * P, :]
                                    .rearrange("(ko p) t -> p ko t", p=P))
                            continue
                        eng = nc.scalar if quar % 2 == 1 else nc.sync
                        eng.dma_start(
                            agt[:, quar * kq:(quar + 1) * kq, :],
                            agc[ci][quar * kq * P:(quar + 1) * kq * P, :]
                            .rearrange("(ko p) t -> p ko t", p=P))
                    return agt

                # chunk 0's panel loads are issued BEFORE the deferred
                # attention floods the scalar queue with exps; otherwise
                # phase C's first matmuls wait ~15us for the scalar-queue
                # half of the panel
                agt0 = load_agt(0, skip_q0=True)
                def_fin = attn_chunk(deferred[0] * NQC + deferred[1],
                                     deferred[0], deferred[1], q_def, False)
                NMD = ODPC // P
                for ci in range(NCHK):
                    tok0 = ci * QCH
                    agt = agt0 if ci == 0 else load_agt(ci)
                    kq = KD // 4
                    # quarter-major accumulation: the first 4*kq matmuls
                    # depend only on the first agt quarter, so phase C
                    # pipelines against the quarter DMAs instead of
                    # waiting for the whole 4MB panel
                    pos = [psab.tile([P, QCH], F32, tag="pa", bufs=4,
                                     name=f"po{ci}_{md}")
                           for md in range(NMD)]
                    for quar in range(4):
                        for md in range(NMD):
                            for kf in range(quar * kq, (quar + 1) * kq):
                                src = (agt0q if (ci == 0 and kf < NPQ)
                                       else agt)
                                nc.tensor.matmul(
                                    pos[md][:],
                                    wo_sb[:, kf, md * P:(md + 1) * P],
                                    src[:, kf, :],
                                    start=(kf == 0), stop=(kf == KD - 1))
                            if quar == 3:
                                # drain each bank as soon as its group stops
                                osb = phco.tile([P, QCH], BF16, tag="osb",
                                                bufs=2, name=f"osb{ci}_{md}")
                                nc.vector.tensor_copy(osb[:], pos[md][:])
                                eng = nc.scalar if (ci >= 1 and md % 2 == 1) \
                                    else nc.sync
                                eng.dma_start(
                                    out_d[md * P:(md + 1) * P,
                                          tok0:tok0 + QCH], osb[:])
                    if ci == 0 and def_fin is not None:
                        def_fin()
                        def_fin = None

    nc.compile()
    return nc


def _host_prep(x, wqkv, wo, sincos, full_causal_mask, start_pos,
               NB, S, D, HPC, NCORES):
    """Shard, cast, and lay out inputs; verify the causal-mask structure."""
    bf16 = ml_dtypes.bfloat16
    TOK = NB * S
    H = HPC * NCORES
    QF = HPC * HD
    NQC = S // QCH
    NKT = S // KT
    KTC = QCH // KT
    ODPC = D // NCORES
    q_sz = H * HD

    # partition-major xt: [p, chunk, ko, t] — per-partition contiguous runs
    # of KD*QCH*2 bytes per chunk so xt DMAs use multi-KB descriptors
    NCHK = TOK // QCH
    KD = D // P
    xt = np.ascontiguousarray(
        x.reshape(NCHK, QCH, KD, P).transpose(3, 0, 2, 1)).astype(bf16)

    # effective mask: [q, k] (batch-shared), incl. the cache-validity term
    m_eff = np.asarray(full_causal_mask[0, 0], dtype=bool)
    m_eff = m_eff[start_pos:start_pos + S, :S].copy()
    valid = np.arange(S) < (start_pos + S)
    m_eff &= valid[None, :]

    # the kernel hardcodes a block-causal structure: block (qc, kt) is full
    # for kt < KTC*qc, diagonal-j (canonical pattern) for kt = KTC*qc + j,
    # empty above.  Verify the provided mask matches.
    qi = np.arange(QCH)
    for qc in range(NQC):
        for kt in range(NKT):
            blk = m_eff[qc * QCH:(qc + 1) * QCH, kt * KT:(kt + 1) * KT]
            j = kt - KTC * qc
            if j < 0:
                assert blk.all(), f"block ({qc},{kt}) expected full"
            elif j < KTC:
                exp_blk = (qi[:, None] >= 128 * j + np.arange(KT)[None, :])
                assert np.array_equal(blk, exp_blk), \
                    f"block ({qc},{kt}) unexpected diagonal pattern"
            else:
                assert not blk.any(), f"block ({qc},{kt}) expected empty"

    # canonical diagonal masks, [k, q] layout, one per j: cols [0,128j) = 0
    # (zeroes the stale prefix), cols [128j, 512): 1 where q >= 128j + k
    maskblk = np.zeros((KTC, KT, QCH), dtype=np.float32)
    for j in range(KTC):
        maskblk[j] = (qi[None, :] >= 128 * j + np.arange(KT)[:, None])
        maskblk[j, :, :128 * j] = 0.0
    maskblk = maskblk.reshape(KTC * KT, QCH).astype(bf16)

    # rope tables, transposed + duplicated halves; sin rows 0:64 negated
    sc = np.asarray(sincos[start_pos:start_pos + S], dtype=np.float32)
    sin, cos = sc[:, :HHD], sc[:, HHD:]
    cosT2 = np.concatenate([cos.T, cos.T], axis=0)           # [128, S]
    sinT2 = np.concatenate([-sin.T, sin.T], axis=0)          # [128, S]
    sincos2 = np.concatenate([cosT2, sinT2], axis=1).astype(bf16)

    in_maps = []
    for c in range(NCORES):
        qcols = np.asarray(wqkv[:, c * QF:(c + 1) * QF])
        kcols = np.asarray(wqkv[:, q_sz + c * HD:q_sz + (c + 1) * HD])
        vcols = np.asarray(
            wqkv[:, q_sz + NCORES * HD + c * HD:
                 q_sz + NCORES * HD + (c + 1) * HD])
        wqkv_c = np.concatenate([qcols, kcols, vcols], axis=1)
        # partition-major weights: [p, ko, cols]
        wqkv_c = np.ascontiguousarray(
            wqkv_c.reshape(KD, P, -1).transpose(1, 0, 2)).astype(bf16)
        wo_c = np.ascontiguousarray(
            np.asarray(wo[:, c * ODPC:(c + 1) * ODPC])
            .reshape(KD, P, ODPC).transpose(1, 0, 2)).astype(bf16)
        in_maps.append({
            "xt": xt, "wqkv": wqkv_c, "wo": wo_c,
            "sincos2": sincos2, "maskblk": maskblk,
        })
    return in_maps


_CACHE = {}


def run_distributed(x, wqkv, wo, sincos, full_causal_mask, start_pos,
                    NB, S, D, HPC, NCORES, trace=False, tmpdir=None):
    in_maps = _host_prep(
        x, wqkv, wo, sincos, full_causal_mask, start_pos,
        NB, S, D, HPC, NCORES)
    key = (NB, S, D, HPC, NCORES)
    if key not in _CACHE:
        _CACHE[key] = build_graph(NB, S, D, HPC, NCORES)
    nc = _CACHE[key]
    res = run_bass_kernel_spmd(nc, in_maps, list(range(NCORES)), trace=trace,
                               tmpdir=tmpdir)
    TOK = NB * S
    out = np.empty((TOK, D), dtype=np.float32)
    ODPC = D // NCORES
    for c in range(NCORES):
        out[:, c * ODPC:(c + 1) * ODPC] = \
            np.asarray(res.results[c]["out"], dtype=np.float32).T
    return out.reshape(NB, S, D), res


def kernel(x, wqkv, wo, sincos, cache_k, cache_v, full_causal_mask,
           start_pos) -> np.ndarray:
    x = np.asarray(x)
    start_pos = int(np.asarray(start_pos))
    B, S_, D_ = x.shape
    assert start_pos == 0, "prefill-only kernel (seq fills the whole cache)"
    out, _ = run_distributed(
        x, np.asarray(wqkv), np.asarray(wo), np.asarray(sincos),
        np.asarray(full_causal_mask), start_pos,
        NB=B, S=S_, D=D_, HPC=4, NCORES=8)
    return out



# revision 36
# speedup vs baseline: 1.0082x; 1.0082x over previous
"""Trainium2 8-core GQA attention kernel (tensor-parallel over heads).

Strategy (8 NeuronCores, SPMD):
  - Core c owns q-heads [4c..4c+4) and kv-head c (GQA groups stay aligned).
  - Phases A (qkv projection + RoPE) and B (attention) are merged per token
    chunk: causality means chunk (b,qc) only attends k-chunks <= qc, so the
    attention for a chunk is emitted right after its projection and the Tile
    scheduler fills attention's exp-latency stalls with projection matmuls.
  - qkvT = wqkv_c^T @ x^T is computed feature-major so Q^T/K^T land in
    [head_dim, tokens] layout; RoPE applied with partition-shifted PSUM
    multiply-adds (bf16 intermediates); V^T -> V via DMA transposes; x^T
    streamed as quarter panels shared by the PSUM m-groups.  Projection runs
    in three m-groups [k,v],[q0,q1],[q2,q3] (chunk 0: two groups of three to
    match the cold DMA ramp): k's rope is emitted first so attention's score
    matmuls never wait on it, and each group's PSUM allocs reuse buffers
    freed a full group earlier (no alloc-wait micro-stalls).
  - Attention scores are computed transposed (S^T[k,q]) so exp(S^T) feeds the
    PV matmul directly (lhsT = V[k,d]) with zero P transposes; fully-masked
    causal blocks are skipped; diagonal blocks restrict score/exp/PV to the
    valid q-range (N = 512-128j) and use a canonical per-j {0,1} mask that
    also zeroes the stale q-prefix; denominators for all 4 heads accumulate
    into one [4, 512] PSUM row-set via indicator-column matmuls, up to 16
    exp-blocks per matmul (summed on DVE), emitted with one-head lag so the
    PE queue never stalls on DVE; 1/den is broadcast across partitions on
    GpSimd (no PE broadcast matmuls); normalization is deferred to the output.
  - A tiny warm-up AllGather issued at t=0 absorbs the one-time collective
    barrier + ncfw cold start (~60us) during the DMA-bound startup; the real
    per-chunk AllGathers then run warm (~25us) and never back-pressure the
    GpSimd queue (whose blocking collective waits would otherwise stall the
    partition_broadcast -> at-multiply -> Vector -> PE chain for ~25us).
  - The AllGather of attention outputs is split into 8 token-chunk
    collectives issued as soon as each chunk's attention completes; phase C
    (out^T = wo_c^T @ attn^T) runs as a solid block at the end.  wo is
    preloaded into SBUF during phase A; the deferred last attention chunk's
    V-transposes ride the then-idle scalar queue; the first NPQ k-tiles of
    phase C's first panel live in a persistent tile prefetched mid-phase-A,
    dodging the SBUF WAR hazard that blocks the pool-allocated panels until
    the last proj chunk retires.
  - Host: inputs are pre-transposed to partition-major DRAM layouts
    ([p, ko, cols] / [p, chunk, ko, t]) so DMA descriptors are multi-KB
    runs; cold start fans the first weight/x loads across the sync, scalar
    AND gpsimd issue queues.
All PE math in bf16 (f32 PSUM accumulation).  HW exec ~900us (from 1151us
baseline): PE is GPIO-power-throttled to ~1.95GHz for ~82% of the span, PE
busy ~93%, remaining idle = cold-start DMA ramp + fixed teardown.
"""

import numpy as np
import ml_dtypes

import concourse.bass as bass
import concourse.mybir as mybir
import concourse.tile as tile
from concourse import bacc
from concourse.bass_utils import run_bass_kernel_spmd

BF16 = mybir.dt.bfloat16
F32 = mybir.dt.float32
HD = 128            # head dim
HHD = HD // 2       # rope half
P = 128             # partitions
QCH = 512           # q-chunk / token-chunk size
KT = 128            # k tile (partition dim)
SCALE = 1.0 / np.sqrt(HD)
DGRP = 16           # exp blocks summed per denominator matmul


def build_graph(NB, S, D, HPC, NCORES):
    """Build the per-core SPMD graph (full-prefill causal schedule)."""
    TOK = NB * S
    QF = HPC * HD           # q features per core
    FLOC = QF + 2 * HD      # local qkv features (q + k + v)
    MT = FLOC // P          # feature tiles (q tiles + 1 k + 1 v)
    KD = D // P             # contraction tiles over model dim
    NQC = S // QCH          # q chunks per batch
    NKT = S // KT           # k tiles per batch
    KTC = QCH // KT         # k tiles per token chunk
    ODPC = D // NCORES      # output dims per core
    NCHK = TOK // QCH       # token chunks overall

    nc = bacc.Bacc("TRN2", target_bir_lowering=False, debug=False,
                   num_devices=NCORES)

    # partition-major DRAM layouts (host pre-transposed): per-partition
    # contiguous runs of several KB -> large DMA descriptors, ~1.5-8x the
    # per-engine transfer efficiency of the naive [D, TOK] layout
    xt_d = nc.dram_tensor("xt", [P, NCHK, KD, QCH], BF16,
                          kind="ExternalInput").ap()
    wqkv_d = nc.dram_tensor("wqkv", [P, KD, FLOC], BF16,
                            kind="ExternalInput").ap()
    wo_d = nc.dram_tensor("wo", [P, KD, ODPC], BF16,
                          kind="ExternalInput").ap()
    sc_d = nc.dram_tensor("sincos2", [P, 2 * S], BF16, kind="ExternalInput").ap()
    mask_d = nc.dram_tensor("maskblk", [KTC * P, QCH], BF16,
                            kind="ExternalInput").ap()
    out_d = nc.dram_tensor("out", [ODPC, TOK], BF16,
                           kind="ExternalOutput").ap()

    with tile.TileContext(nc) as tc:
        with tc.tile_pool(name="persist", bufs=1) as persist, \
             tc.tile_pool(name="dram", bufs=1, space="DRAM") as dram:
            kT = persist.tile([P, TOK], BF16)          # K^T, all tokens
            q_def = persist.tile([P, HPC, QCH], BF16)  # deferred chunk's Q^T
            v_kd = persist.tile([P, NB * NKT, HD], BF16)
            wq_sb = persist.tile([P, KD, FLOC], BF16)
            wo_sb = persist.tile([P, KD, ODPC], BF16)
            sc_sb = persist.tile([P, 2 * S], BF16)
            mk = persist.tile([P, KTC, QCH], BF16)     # canonical diag masks
            # indicator columns for per-head denominator batching
            ecol = persist.tile([P, HPC, HPC], BF16)   # [:, h, :] = e_h cols
            nc.vector.memset(ecol[:], 0.0)
            for h in range(HPC):
                nc.vector.memset(ecol[:, h, h:h + 1], 1.0)

            # phase-C chunk 0's first xt-panel quarter, persistent: loaded
            # mid-phase-A (agc[0] is ready ~450us before phase C) so the
            # first wo matmuls never wait on the SBUF WAR hazard that blocks
            # the pool-allocated panel until the last proj chunk retires
            NPQ = 6            # k-tiles of chunk-0's first quarter kept persistent
            agt0q = persist.tile([P, NPQ, QCH], BF16)

            cosT = sc_sb[:, 0:S]
            sinT = sc_sb[:, S:2 * S]

            bounce = [dram.tile([QF, QCH], BF16, name=f"bnc{ci}")
                      for ci in range(NCHK)]
            agc = [dram.tile([QF * NCORES, QCH], BF16, name=f"agc{ci}",
                             addr_space="Shared" if NCORES > 4 else "Local")
                   for ci in range(NCHK)]
            # tiny warm-up AllGather issued first: absorbs the one-time
            # collective barrier + ncfw cold-start (~60us) during the
            # DMA-bound startup, so the first REAL AllGather runs at warm
            # speed (~24us) and never back-pressures the GpSimd queue
            warm_in = dram.tile([1, 64], BF16, name="warm_in")
            warm_out = dram.tile([NCORES, 64], BF16, name="warm_out",
                                 addr_space="Shared" if NCORES > 4 else "Local")

            # ---------- merged phases A (projection+RoPE) and B (attention) --
            with tc.tile_pool(name="phbw", bufs=3) as phbw, \
                 tc.tile_pool(name="psab", bufs=1, space="PSUM") as psab:
              with tc.tile_pool(name="phax", bufs=2) as phax, \
                 tc.tile_pool(name="phaq", bufs=2) as phaq, \
                 tc.tile_pool(name="phat", bufs=2) as phat:

                KQ = KD // 4        # k-tiles per xt quarter

                def load_xt_quarter(col0, quar, tagname, nspl=1):
                    """One quarter of a chunk's x^T panel; the four quarter
                    tiles are shared by all m-groups (no per-group reloads)
                    and rotate a 6-buffer pool so the next chunk's loads
                    overlap this chunk's last-group reads."""
                    xt_sb = phax.tile([P, KQ, QCH], BF16, tag="xt", bufs=5,
                                      name=tagname)
                    ch = col0 // QCH
                    qk = KQ // nspl
                    for q4 in range(nspl):
                        nc.sync.dma_start(
                            xt_sb[:, q4 * qk:(q4 + 1) * qk, :],
                            xt_d[:, ch,
                                 quar * KQ + q4 * qk:
                                 quar * KQ + (q4 + 1) * qk, :])
                    return xt_sb

                # interleave the critical first loads across both queues;
                # after startup the scalar queue carries ONLY exp activations
                # (a 600ns DMA-descriptor issue ahead of an exp delays the
                # whole attention pipeline on the in-order scalar queue)
                # cold start: three issue queues run in parallel.  k0 is
                # split across sync+scalar (first matmul gates on it); wq
                # k1-k7 ride the otherwise-idle gpsimd queue (emitted BEFORE
                # the warm-up AllGather, which then blocks that queue for
                # ~70us); xt quarter-0 subloads alternate sync/scalar in
                # consumption order.
                # cold start: three issue queues in parallel.  k0 is split
                # across sync+scalar (the first matmul gates on it); wq
                # k1-k7 ride the otherwise-idle gpsimd queue (software DMA
                # path, ~18GB/s, emitted BEFORE the warm-up AllGather which
                # then blocks that queue); xt quarter-0 subloads alternate
                # sync/scalar in consumption order.
                HFL = FLOC // 2
                nc.scalar.dma_start(wq_sb[:, 0, 0:HFL], wqkv_d[:, 0, 0:HFL])
                nc.sync.dma_start(wq_sb[:, 0, HFL:FLOC],
                                  wqkv_d[:, 0, HFL:FLOC])
                for ko in range(1, 8):
                    nc.gpsimd.dma_start(wq_sb[:, ko, :], wqkv_d[:, ko, :])
                nc.gpsimd.collective_compute(
                    "AllGather", mybir.AluOpType.bypass,
                    replica_groups=[list(range(NCORES))],
                    ins=[warm_in.opt()], outs=[warm_out.opt()])
                xtf0 = phax.tile([P, KQ, QCH], BF16, tag="xt", bufs=5,
                                 name="xtf0")
                for a, b, eng in ((0, 1, nc.sync), (1, 2, nc.scalar),
                                  (2, 4, nc.sync), (4, 6, nc.scalar),
                                  (6, 8, nc.sync)):
                    eng.dma_start(xtf0[:, a:b, :], xt_d[:, 0, a:b, :])
                xt_q0 = [xtf0]
                # rope tables for position range [0,512) (used by chunk 0 of
                # both batches), then the rest
                nc.scalar.dma_start(sc_sb[:, 0:QCH], sc_d[:, 0:QCH])
                nc.scalar.dma_start(sc_sb[:, S:S + QCH], sc_d[:, S:S + QCH])

                # weight feed split across both queues; early tiles go
                # individually (arrival order matches the first chunk's
                # kg-order consumption), later ones as multi-tile range
                # DMAs whose per-partition-contiguous descriptors run at
                # full engine rate
                def wq_span(k0, k1, step=1):
                    for ko in range(k0, k1, step):
                        hi = min(ko + step, k1)
                        eng = nc.sync if (ko // step) % 2 == 0 else nc.scalar
                        eng.dma_start(
                            wq_sb[:, ko:hi, :], wqkv_d[:, ko:hi, :])

                xt_q0.append(load_xt_quarter(0, 1, "xtf1", nspl=2))
                wq_span(8, 15, step=2)
                xt_q0.append(load_xt_quarter(0, 2, "xtf2", nspl=2))
                wq_span(15, 23, step=4)
                xt_q0.append(load_xt_quarter(0, 3, "xtf3"))
                wq_span(23, KD, step=4)
                # diag masks + rest of rope tables
                nc.sync.dma_start(
                    mk[:], mask_d[:].rearrange("(mb p) q -> p mb q", p=P))
                nc.sync.dma_start(sc_sb[:, QCH:S], sc_d[:, QCH:S])
                nc.sync.dma_start(sc_sb[:, S + QCH:2 * S],
                                  sc_d[:, S + QCH:2 * S])

                # k and v ride in the FIRST group: attention's score matmuls
                # need this chunk's roped kT (emitting k's rope first means it
                # completes ~20us before attention reaches the PE queue,
                # instead of being the very last DVE op).  Three groups of 2
                # also mean each group's PSUM allocs reuse buffers freed
                # early in the group-before-last — no alloc-wait micro-stalls
                # (2 live + 2 next = 4 bufs exactly).  Chunk 0 instead uses
                # two groups of 3: a 2-wide group consumes the xt panel at
                # ~240GB/s, outpacing the cold DMA ramp — 3-wide matches it.
                m_groups = [[MT - 2, MT - 1], [0, 1], [2, 3]]
                m_groups0 = [[MT - 2, MT - 1, 0], [1, 2, 3]]

                def rope_store(pss, dst, s0):
                    """dst(bf16) = pss*cos + rot_half(pss)*sin.

                    The multiplies read PSUM f32 (1 elem/cycle on DVE — a
                    partition-shifted tensor_tensor needs one non-SBUF
                    input anyway); t1/t2 are bf16 so the final add runs at
                    2 elem/cycle.  DVE cost ~2.5us per tile.
                    """
                    t1 = phat.tile([P, QCH], BF16, tag="t1",
                                   name=f"t1_{rope_store.n}")
                    t2 = phat.tile([P, QCH], BF16, tag="t2",
                                   name=f"t2_{rope_store.n}")
                    nc.vector.tensor_mul(t1[:], pss[:], cosT[:, s0:s0 + QCH])
                    nc.vector.tensor_mul(t2[0:HHD, :], pss[HHD:P, :],
                                         sinT[0:HHD, s0:s0 + QCH])
                    nc.vector.tensor_mul(t2[HHD:P, :], pss[0:HHD, :],
                                         sinT[HHD:P, s0:s0 + QCH])
                    nc.vector.tensor_add(dst, t1[:], t2[:])
                    rope_store.n += 1
                rope_store.n = 0

                def proj_filler(b, cb, q_dst):
                    """Chunk (b,cb)'s projection as a filler: fill(n) emits
                    up to n matmuls (returns False when exhausted), with the
                    group-boundary DVE work (ropes / v copy / transposes)
                    emitted as the generator crosses each group.  The
                    previous chunk's attention weaves these matmuls between
                    its exp-paced score/PV pairs, so the PE never drains
                    while the scalar engine catches up."""
                    ch = b * NQC + cb
                    col0 = ch * QCH
                    s0 = col0 % S
                    vT = phaq.tile([P, QCH], BF16, tag="vt", name=f"vT{ch}")
                    # quarters 0/1 load eagerly; 2/3 lazily (emitted when
                    # the generator enters the previous quarter).  An
                    # eagerly-emitted quarter DMA carries a WAR wait on the
                    # previous chunk's last reads — with the weave those
                    # now happen much later, and that wait would block the
                    # whole sync queue head-of-line (measured 35us stall).
                    xtq = list(xt_q0) if ch == 0 else [None] * 4

                    def get_q(quar):
                        if xtq[quar] is None:
                            xtq[quar] = load_xt_quarter(
                                col0, quar, f"xt{ch}_{quar}")
                        return xtq[quar]
                    if ch != 0:
                        get_q(0)
                        get_q(1)

                    def dve_for(grp, pss):
                        # emit DVE consumers in group-list order (k first)
                        for m in grp:
                            if m == MT - 1:          # v
                                nc.vector.tensor_copy(vT[:], pss[m][:])
                                # V^T -> V transposes: sync queue normally
                                # (a scalar-queue DMA_TRANSPOSE costs 1.2us
                                # of queue occupancy right where attention's
                                # exps live).  The DEFERRED chunk's
                                # attention runs much later (phase C), so
                                # its transposes go on the then-idle scalar
                                # queue instead of behind ~4MB of xt loads
                                # on sync (which starved its PV by ~7us).
                                eng_t = (nc.scalar if ch == NB * NQC - 1
                                         else nc.sync)
                                for j in range(KTC):
                                    kt = cb * KTC + j
                                    eng_t.dma_start_transpose(
                                        v_kd[:, b * NKT + kt, :],
                                        vT[:, j * KT:(j + 1) * KT])
                            elif m == MT - 2:        # k
                                rope_store(pss[m], kT[:, col0:col0 + QCH],
                                           s0)
                            else:                    # q
                                rope_store(pss[m], q_dst[:, m, :], s0)

                    def gen():
                        for grp in (m_groups0 if ch == 0 else m_groups):
                            pss = {m: psab.tile([P, QCH], F32, tag="pa",
                                                bufs=4, name=f"pa{ch}_{m}")
                                   for m in grp}
                            for kg in range(KD):
                                quar = kg // KQ
                                xt_sb = get_q(quar)
                                if kg % KQ == 0 and quar < 3:
                                    get_q(quar + 1)
                                for m in grp:
                                    nc.tensor.matmul(
                                        pss[m][:],
                                        wq_sb[:, kg, m * P:(m + 1) * P],
                                        xt_sb[:, kg % KQ, :],
                                        start=(kg == 0),
                                        stop=(kg == KD - 1))
                                    yield True
                            dve_for(grp, pss)

                    g = gen()

                    def fill(n=1):
                        for _ in range(n):
                            if next(g, None) is None:
                                return False
                        return True
                    return fill

                def attn_chunk(ci, b, qc, q_t, full_exp, fill=None):
                    kts = list(range(KTC * (qc + 1)))   # causal prefill
                    # d_ps lives in the "st" pool (scores then run with a
                    # 2-deep lookahead — enough, because fill() weaves proj
                    # matmuls between each score and its exp-gated PV)
                    d_ps = psab.tile([HPC, QCH], F32, tag="st", bufs=3,
                                     name=f"den{ci}")
                    o_tiles = {}
                    n_dgrp = (len(kts) + DGRP - 1) // DGRP
                    dtot = n_dgrp * HPC
                    dcnt = 0
                    step = 0
                    pending = []      # (step_ready, head, dacc)

                    def emit_denoms(lag):
                        nonlocal dcnt
                        while pending and step - pending[0][0] >= lag:
                            _, dh, dacc = pending.pop(0)
                            nc.tensor.matmul(
                                d_ps[:], ecol[:, dh, :], dacc[:],
                                start=(dcnt == 0), stop=(dcnt == dtot - 1))
                            dcnt += 1

                    for h in range(HPC):
                        o_ps = psab.tile([P, QCH], F32, tag="outT", bufs=1,
                                         name=f"o{ci}_{h}")
                        dacc = None
                        dacc_n = 0
                        for i, kt in enumerate(kts):
                            emit_denoms(lag=3)
                            j = kt - KTC * qc
                            qlo = 128 * j if (j >= 1 and not full_exp) else 0
                            st = psab.tile([P, QCH], F32, tag="st", bufs=3,
                                           name=f"st{ci}_{h}_{i}")
                            nc.tensor.matmul(
                                st[:, qlo:QCH],
                                kT[:, b * S + kt * KT:b * S + (kt + 1) * KT],
                                q_t[:, h, qlo:QCH],
                                start=True, stop=True)
                            pt = phbw.tile([P, QCH], BF16, tag="pt", bufs=7,
                                           name=f"pt{ci}_{h}_{i}")
                            nc.scalar.activation(
                                pt[:, qlo:QCH], st[:, qlo:QCH],
                                mybir.ActivationFunctionType.Exp,
                                bias=0.0, scale=float(SCALE))
                            if j >= 0:   # diagonal: mask + zero stale prefix
                                mhi = min(128 * (j + 1), QCH)
                                nc.vector.tensor_mul(pt[:, 0:mhi],
                                                     pt[:, 0:mhi],
                                                     mk[:, j, 0:mhi])
                            if fill is not None:
                                # weave proj matmuls of the NEXT chunk here:
                                # the PE covers this block's exp (+mask)
                                # latency with useful work instead of
                                # idling in-order behind the PV
                                fill(3 if j >= 0 else 2)
                            first, last = (i == 0), (i == len(kts) - 1)
                            nc.tensor.matmul(
                                o_ps[:, qlo:QCH],
                                v_kd[:, b * NKT + kt, :], pt[:, qlo:QCH],
                                start=first, stop=last)
                            # batch up to DGRP exp blocks per denom matmul;
                            # diagonal blocks only contribute on [qlo:] (the
                            # prefix is masked to zero), so restrict the add
                            gpos = i % DGRP
                            if gpos == 0:
                                dacc, dacc_n = pt, 1
                            else:
                                if dacc_n == 1:
                                    dsum = phbw.tile([P, QCH], BF16,
                                                     tag="dsum", bufs=2,
                                                     name=f"ds{ci}_{h}_{i}")
                                    nc.vector.tensor_add(dsum[:], dacc[:],
                                                         pt[:])
                                    dacc = dsum
                                else:
                                    nc.vector.tensor_add(dacc[:, qlo:QCH],
                                                         dacc[:, qlo:QCH],
                                                         pt[:, qlo:QCH])
                                dacc_n += 1
                            if gpos == DGRP - 1 or last:
                                pending.append((step, h, dacc))
                            step += 1
                        o_sb = phbw.tile([P, QCH], BF16, tag="osbuf", bufs=5,
                                         name=f"ou{ci}_{h}")
                        nc.vector.tensor_copy(o_sb[:], o_ps[:])
                        o_tiles[h] = o_sb
                        if fill is not None:
                            fill(2)
                    # flush any remaining denominator matmuls
                    step += 1000
                    emit_denoms(lag=0)
                    assert dcnt == dtot, (dcnt, dtot)
                    inv = phbw.tile([HPC, QCH], F32, tag="inv", bufs=1,
                                    name=f"inv{ci}")
                    nc.vector.reciprocal(inv[:], d_ps[:])
                    invb = phbw.tile([HPC, QCH], BF16, tag="invb", bufs=1,
                                     name=f"invb{ci}")
                    nc.vector.tensor_copy(invb[:], inv[:])
                    # flatten the 4 inv rows onto partition 0 (DMA crosses
                    # partitions freely) so partition_broadcast can expand
                    # them; on the scalar queue, where a short wait on invb
                    # blocks nothing (next exps are ~15us away)
                    invf = phbw.tile([1, HPC * QCH], BF16, tag="invf",
                                     bufs=1, name=f"invf{ci}")
                    nc.scalar.dma_start(invf[:], invb[:])

                    def finalize():
                        # normalization tail; emitted mid-way through the
                        # NEXT projection chunk (before its ropes join the
                        # vector queue).  The inv broadcast runs on GpSimd
                        # (otherwise idle) instead of PE matmuls.
                        for h in range(HPC):
                            bcc = phbw.tile([P, QCH], BF16, tag="bcc",
                                            bufs=1, name=f"bcc{ci}_{h}")
                            nc.gpsimd.partition_broadcast(
                                bcc[:], invf[:, h * QCH:(h + 1) * QCH])
                            at = phbw.tile([P, QCH], BF16, tag="at", bufs=1,
                                           name=f"at{ci}_{h}")
                            nc.vector.tensor_mul(at[:], o_tiles[h][:],
                                                 bcc[:])
                            nc.sync.dma_start(
                                bounce[ci][h * P:(h + 1) * P, :], at[:])
                        nc.gpsimd.collective_compute(
                            "AllGather", mybir.AluOpType.bypass,
                            replica_groups=[list(range(NCORES))],
                            ins=[bounce[ci].opt()], outs=[agc[ci].opt()])
                    return finalize

                # full-prefill causal schedule: attention for chunk qc is
                # WOVEN into the next chunk's projection (its fill matmuls
                # hide the exp chain); the very last attention chunk is
                # deferred past the proj-pool close so phase C's first
                # matmuls can fill its exp-latency stalls
                deferred = (NB - 1, NQC - 1)
                wo_sched = {1: (0, 5), 2: (5, 10), 3: (10, 14),
            4: (14, 19), 5: (19, 23), 6: (23, 28),
            7: (28, KD)}
                chunk_no = 0
                prev = None       # previous chunk's attention, not yet run
                for b in range(NB):
                    for cb in range(NQC):
                        if (b, cb) == deferred:
                            q_dst = q_def
                        else:
                            q_dst = phaq.tile([P, HPC, QCH], BF16, tag="qch",
                                              name=f"q{b}_{cb}")
                        fill = proj_filler(b, cb, q_dst)
                        if prev is not None:
                            fin = attn_chunk(*prev, fill=fill)
                            # normalization + AllGather for the chunk that
                            # just finished its attention
                            fin()
                        while fill(16):
                            pass
                        # trickle wo weight loads behind the xt streams
                        chunk_no += 1
                        if chunk_no in wo_sched:
                            lo, hi = wo_sched[chunk_no]
                            nc.sync.dma_start(wo_sb[:, lo:hi, :],
                                              wo_d[:, lo:hi, :])
                        if chunk_no == 4:
                            # prefetch phase-C chunk 0's first panel quarter
                            # into its persistent home (AG0 finished long
                            # ago; both queues have slack mid-phase)
                            nc.sync.dma_start(
                                agt0q[:],
                                agc[0][0:NPQ * P, :]
                                .rearrange("(ko p) t -> p ko t", p=P))
                        if (b, cb) != deferred:
                            ci = b * NQC + cb
                            prev = (ci, b, cb, q_dst,
                                    b == 0 and cb == 0)
                        else:
                            prev = None

              # -------------- Phase C: out^T = wo_c^T @ attn^T ---------------
              with tc.tile_pool(name="phcx", bufs=2) as phcx, \
                 tc.tile_pool(name="phco", bufs=2) as phco:
                def load_agt(ci, skip_q0=False):
                    agt = phcx.tile([P, KD, QCH], BF16, tag="agt",
                                    name=f"agt{ci}")
                    kq = KD // 4
                    for quar in range(4):
                        # first quarter split across both queues so the
                        # first matmuls start as early as possible
                        if quar == 0:
                            if skip_q0:
                                # k-tiles [0:NPQ) served by the persistent
                                # agt0q; load only the remainder
                                nc.sync.dma_start(
                                    agt[:, NPQ:kq, :],
                                    agc[ci][NPQ * P:kq * P, :]
                                    .rearrange("(ko p) t -> p ko t", p=P))
                                continue
                            h = kq // 2
                            for sub, eng in ((0, nc.sync), (1, nc.scalar)):
                                eng.dma_start(
                                    agt[:, sub * h:(sub + 1) * h, :],
                                    agc[ci][sub * h * P:(sub + 1) * h * P, :]
                                    .rearrange("(ko p) t -> p ko t", p=P))
                            continue
                        eng = nc.scalar if quar % 2 == 1 else nc.sync
                        eng.dma_start(
                            agt[:, quar * kq:(quar + 1) * kq, :],
                            agc[ci][quar * kq * P:(quar + 1) * kq * P, :]
                            .rearrange("(ko p) t -> p ko t", p=P))
                    return agt

                # chunk 0's panel loads are issued BEFORE the deferred
                # attention floods the scalar queue with exps; otherwise
                # phase C's first matmuls wait ~15us for the scalar-queue
                # half of the panel
                agt0 = load_agt(0, skip_q0=True)
                def_fin = attn_chunk(deferred[0] * NQC + deferred[1],
                                     deferred[0], deferred[1], q_def, False)
                NMD = ODPC // P
                for ci in range(NCHK):
                    tok0 = ci * QCH
                    agt = agt0 if ci == 0 else load_agt(ci)
                    kq = KD // 4
                    # quarter-major accumulation: the first 4*kq matmuls
                    # depend only on the first agt quarter, so phase C
                    # pipelines against the quarter DMAs instead of
                    # waiting for the whole 4MB panel
                    pos = [psab.tile([P, QCH], F32, tag="pa", bufs=4,
                                     name=f"po{ci}_{md}")
                           for md in range(NMD)]
                    for quar in range(4):
                        for md in range(NMD):
                            for kf in range(quar * kq, (quar + 1) * kq):
                                src = (agt0q if (ci == 0 and kf < NPQ)
                                       else agt)
                                nc.tensor.matmul(
                                    pos[md][:],
                                    wo_sb[:, kf, md * P:(md + 1) * P],
                                    src[:, kf, :],
                                    start=(kf == 0), stop=(kf == KD - 1))
                            if quar == 3:
                                # drain each bank as soon as its group stops
                                osb = phco.tile([P, QCH], BF16, tag="osb",
                                                bufs=2, name=f"osb{ci}_{md}")
                                nc.vector.tensor_copy(osb[:], pos[md][:])
                                eng = nc.scalar if (ci >= 1 and md % 2 == 1) \
                                    else nc.sync
                                eng.dma_start(
                                    out_d[md * P:(md + 1) * P,
                                          tok0:tok0 + QCH], osb[:])
                    if ci == 0 and def_fin is not None:
                        def_fin()
                        def_fin = None

    nc.compile()
    return nc


def _host_prep(x, wqkv, wo, sincos, full_causal_mask, start_pos,
               NB, S, D, HPC, NCORES):
    """Shard, cast, and lay out inputs; verify the causal-mask structure."""
    bf16 = ml_dtypes.bfloat16
    TOK = NB * S
    H = HPC * NCORES
    QF = HPC * HD
    NQC = S // QCH
    NKT = S // KT
    KTC = QCH // KT
    ODPC = D // NCORES
    q_sz = H * HD

    # partition-major xt: [p, chunk, ko, t] — per-partition contiguous runs
    # of KD*QCH*2 bytes per chunk so xt DMAs use multi-KB descriptors
    NCHK = TOK // QCH
    KD = D // P
    xt = np.ascontiguousarray(
        x.reshape(NCHK, QCH, KD, P).transpose(3, 0, 2, 1)).astype(bf16)

    # effective mask: [q, k] (batch-shared), incl. the cache-validity term
    m_eff = np.asarray(full_causal_mask[0, 0], dtype=bool)
    m_eff = m_eff[start_pos:start_pos + S, :S].copy()
    valid = np.arange(S) < (start_pos + S)
    m_eff &= valid[None, :]

    # the kernel hardcodes a block-causal structure: block (qc, kt) is full
    # for kt < KTC*qc, diagonal-j (canonical pattern) for kt = KTC*qc + j,
    # empty above.  Verify the provided mask matches.
    qi = np.arange(QCH)
    for qc in range(NQC):
        for kt in range(NKT):
            blk = m_eff[qc * QCH:(qc + 1) * QCH, kt * KT:(kt + 1) * KT]
            j = kt - KTC * qc
            if j < 0:
                assert blk.all(), f"block ({qc},{kt}) expected full"
            elif j < KTC:
                exp_blk = (qi[:, None] >= 128 * j + np.arange(KT)[None, :])
                assert np.array_equal(blk, exp_blk), \
                    f"block ({qc},{kt}) unexpected diagonal pattern"
            else:
                assert not blk.any(), f"block ({qc},{kt}) expected empty"

    # canonical diagonal masks, [k, q] layout, one per j: cols [0,128j) = 0
    # (zeroes the stale prefix), cols [128j, 512): 1 where q >= 128j + k
    maskblk = np.zeros((KTC, KT, QCH), dtype=np.float32)
    for j in range(KTC):
        maskblk[j] = (qi[None, :] >= 128 * j + np.arange(KT)[:, None])
        maskblk[j, :, :128 * j] = 0.0
    maskblk = maskblk.reshape(KTC * KT, QCH).astype(bf16)

    # rope tables, transposed + duplicated halves; sin rows 0:64 negated
    sc = np.asarray(sincos[start_pos:start_pos + S], dtype=np.float32)
    sin, cos = sc[:, :HHD], sc[:, HHD:]
    cosT2 = np.concatenate([cos.T, cos.T], axis=0)           # [128, S]
    sinT2 = np.concatenate([-sin.T, sin.T], axis=0)          # [128, S]
    sincos2 = np.concatenate([cosT2, sinT2], axis=1).astype(bf16)

    in_maps = []
    for c in range(NCORES):
        qcols = np.asarray(wqkv[:, c * QF:(c + 1) * QF])
        kcols = np.asarray(wqkv[:, q_sz + c * HD:q_sz + (c + 1) * HD])
        vcols = np.asarray(
            wqkv[:, q_sz + NCORES * HD + c * HD:
                 q_sz + NCORES * HD + (c + 1) * HD])
        wqkv_c = np.concatenate([qcols, kcols, vcols], axis=1)
        # partition-major weights: [p, ko, cols]
        wqkv_c = np.ascontiguousarray(
            wqkv_c.reshape(KD, P, -1).transpose(1, 0, 2)).astype(bf16)
        wo_c = np.ascontiguousarray(
            np.asarray(wo[:, c * ODPC:(c + 1) * ODPC])
            .reshape(KD, P, ODPC).transpose(1, 0, 2)).astype(bf16)
        in_maps.append({
            "xt": xt, "wqkv": wqkv_c, "wo": wo_c,
            "sincos2": sincos2, "maskblk": maskblk,
        })
    return in_maps


_CACHE = {}


def run_distributed(x, wqkv, wo, sincos, full_causal_mask, start_pos,
                    NB, S, D, HPC, NCORES, trace=False, tmpdir=None):
    in_maps = _host_prep(
        x, wqkv, wo, sincos, full_causal_mask, start_pos,
        NB, S, D, HPC, NCORES)
    key = (NB, S, D, HPC, NCORES)
    if key not in _CACHE:
        _CACHE[key] = build_graph(NB, S, D, HPC, NCORES)
    nc = _CACHE[key]
    res = run_bass_kernel_spmd(nc, in_maps, list(range(NCORES)), trace=trace,
                               tmpdir=tmpdir)
    TOK = NB * S
    out = np.empty((TOK, D), dtype=np.float32)
    ODPC = D // NCORES
    for c in range(NCORES):
        out[:, c * ODPC:(c + 1) * ODPC] = \
            np.asarray(res.results[c]["out"], dtype=np.float32).T
    return out.reshape(NB, S, D), res


def kernel(x, wqkv, wo, sincos, cache_k, cache_v, full_causal_mask,
           start_pos) -> np.ndarray:
    x = np.asarray(x)
    start_pos = int(np.asarray(start_pos))
    B, S_, D_ = x.shape
    assert start_pos == 0, "prefill-only kernel (seq fills the whole cache)"
    out, _ = run_distributed(
        x, np.asarray(wqkv), np.asarray(wo), np.asarray(sincos),
        np.asarray(full_causal_mask), start_pos,
        NB=B, S=S_, D=D_, HPC=4, NCORES=8)
    return out



# revision 37
# speedup vs baseline: 1.0153x; 1.0071x over previous
"""Trainium2 8-core GQA attention kernel (tensor-parallel over heads).

Strategy (8 NeuronCores, SPMD):
  - Core c owns q-heads [4c..4c+4) and kv-head c (GQA groups stay aligned).
  - Phases A (qkv projection + RoPE) and B (attention) are merged per token
    chunk: causality means chunk (b,qc) only attends k-chunks <= qc, so the
    attention for a chunk is emitted right after its projection and the Tile
    scheduler fills attention's exp-latency stalls with projection matmuls.
  - qkvT = wqkv_c^T @ x^T is computed feature-major so Q^T/K^T land in
    [head_dim, tokens] layout; RoPE applied with partition-shifted PSUM
    multiply-adds (bf16 intermediates); V^T -> V via DMA transposes; x^T
    streamed as quarter panels shared by the PSUM m-groups.  Projection runs
    in three m-groups [k,v],[q0,q1],[q2,q3] (chunk 0: two groups of three to
    match the cold DMA ramp): k's rope is emitted first so attention's score
    matmuls never wait on it, and each group's PSUM allocs reuse buffers
    freed a full group earlier (no alloc-wait micro-stalls).
  - Attention scores are computed transposed (S^T[k,q]) so exp(S^T) feeds the
    PV matmul directly (lhsT = V[k,d]) with zero P transposes; fully-masked
    causal blocks are skipped; diagonal blocks restrict score/exp/PV to the
    valid q-range (N = 512-128j) and use a canonical per-j {0,1} mask that
    also zeroes the stale q-prefix; denominators for all 4 heads accumulate
    into one [4, 512] PSUM row-set via indicator-column matmuls, up to 16
    exp-blocks per matmul (summed on DVE), emitted with one-head lag so the
    PE queue never stalls on DVE; 1/den is broadcast across partitions on
    GpSimd (no PE broadcast matmuls); normalization is deferred to the output.
  - A tiny warm-up AllGather issued at t=0 absorbs the one-time collective
    barrier + ncfw cold start (~60us) during the DMA-bound startup; the real
    per-chunk AllGathers then run warm (~25us) and never back-pressure the
    GpSimd queue (whose blocking collective waits would otherwise stall the
    partition_broadcast -> at-multiply -> Vector -> PE chain for ~25us).
  - The AllGather of attention outputs is split into 8 token-chunk
    collectives issued as soon as each chunk's attention completes; phase C
    (out^T = wo_c^T @ attn^T) runs as a solid block at the end.  wo is
    preloaded into SBUF during phase A; the deferred last attention chunk's
    V-transposes ride the then-idle scalar queue; the first NPQ k-tiles of
    phase C's first panel live in a persistent tile prefetched mid-phase-A,
    dodging the SBUF WAR hazard that blocks the pool-allocated panels until
    the last proj chunk retires.
  - Host: inputs are pre-transposed to partition-major DRAM layouts
    ([p, ko, cols] / [p, chunk, ko, t]) so DMA descriptors are multi-KB
    runs; cold start fans the first weight/x loads across the sync, scalar
    AND gpsimd issue queues.
All PE math in bf16 (f32 PSUM accumulation).  HW exec ~900us (from 1151us
baseline): PE is GPIO-power-throttled to ~1.95GHz for ~82% of the span, PE
busy ~93%, remaining idle = cold-start DMA ramp + fixed teardown.
"""

import numpy as np
import ml_dtypes

import concourse.bass as bass
import concourse.mybir as mybir
import concourse.tile as tile
from concourse import bacc
from concourse.bass_utils import run_bass_kernel_spmd

BF16 = mybir.dt.bfloat16
F32 = mybir.dt.float32
HD = 128            # head dim
HHD = HD // 2       # rope half
P = 128             # partitions
QCH = 512           # q-chunk / token-chunk size
KT = 128            # k tile (partition dim)
SCALE = 1.0 / np.sqrt(HD)
DGRP = 16           # exp blocks summed per denominator matmul


def build_graph(NB, S, D, HPC, NCORES):
    """Build the per-core SPMD graph (full-prefill causal schedule)."""
    TOK = NB * S
    QF = HPC * HD           # q features per core
    FLOC = QF + 2 * HD      # local qkv features (q + k + v)
    MT = FLOC // P          # feature tiles (q tiles + 1 k + 1 v)
    KD = D // P             # contraction tiles over model dim
    NQC = S // QCH          # q chunks per batch
    NKT = S // KT           # k tiles per batch
    KTC = QCH // KT         # k tiles per token chunk
    ODPC = D // NCORES      # output dims per core
    NCHK = TOK // QCH       # token chunks overall

    nc = bacc.Bacc("TRN2", target_bir_lowering=False, debug=False,
                   num_devices=NCORES)

    # partition-major DRAM layouts (host pre-transposed): per-partition
    # contiguous runs of several KB -> large DMA descriptors, ~1.5-8x the
    # per-engine transfer efficiency of the naive [D, TOK] layout
    xt_d = nc.dram_tensor("xt", [P, NCHK, KD, QCH], BF16,
                          kind="ExternalInput").ap()
    wqkv_d = nc.dram_tensor("wqkv", [P, KD, FLOC], BF16,
                            kind="ExternalInput").ap()
    wo_d = nc.dram_tensor("wo", [P, KD, ODPC], BF16,
                          kind="ExternalInput").ap()
    sc_d = nc.dram_tensor("sincos2", [P, 2 * S], BF16, kind="ExternalInput").ap()
    mask_d = nc.dram_tensor("maskblk", [KTC * P, QCH], BF16,
                            kind="ExternalInput").ap()
    out_d = nc.dram_tensor("out", [ODPC, TOK], BF16,
                           kind="ExternalOutput").ap()

    with tile.TileContext(nc) as tc:
        with tc.tile_pool(name="persist", bufs=1) as persist, \
             tc.tile_pool(name="dram", bufs=1, space="DRAM") as dram:
            kT = persist.tile([P, TOK], BF16)          # K^T, all tokens
            q_def = persist.tile([P, HPC, QCH], BF16)  # deferred chunk's Q^T
            v_kd = persist.tile([P, NB * NKT, HD], BF16)
            wq_sb = persist.tile([P, KD, FLOC], BF16)
            wo_sb = persist.tile([P, KD, ODPC], BF16)
            sc_sb = persist.tile([P, 2 * S], BF16)
            mk = persist.tile([P, KTC, QCH], BF16)     # canonical diag masks
            # indicator columns for per-head denominator batching
            ecol = persist.tile([P, HPC, HPC], BF16)   # [:, h, :] = e_h cols
            nc.vector.memset(ecol[:], 0.0)
            for h in range(HPC):
                nc.vector.memset(ecol[:, h, h:h + 1], 1.0)

            # phase-C chunk 0's first xt-panel quarter, persistent: loaded
            # mid-phase-A (agc[0] is ready ~450us before phase C) so the
            # first wo matmuls never wait on the SBUF WAR hazard that blocks
            # the pool-allocated panel until the last proj chunk retires
            NPQ = 6            # k-tiles of chunk-0's first quarter kept persistent
            agt0q = persist.tile([P, NPQ, QCH], BF16)

            cosT = sc_sb[:, 0:S]
            sinT = sc_sb[:, S:2 * S]

            bounce = [dram.tile([QF, QCH], BF16, name=f"bnc{ci}")
                      for ci in range(NCHK)]
            agc = [dram.tile([QF * NCORES, QCH], BF16, name=f"agc{ci}",
                             addr_space="Shared" if NCORES > 4 else "Local")
                   for ci in range(NCHK)]
            # tiny warm-up AllGather issued first: absorbs the one-time
            # collective barrier + ncfw cold-start (~60us) during the
            # DMA-bound startup, so the first REAL AllGather runs at warm
            # speed (~24us) and never back-pressures the GpSimd queue
            warm_in = dram.tile([1, 64], BF16, name="warm_in")
            warm_out = dram.tile([NCORES, 64], BF16, name="warm_out",
                                 addr_space="Shared" if NCORES > 4 else "Local")

            # ---------- merged phases A (projection+RoPE) and B (attention) --
            with tc.tile_pool(name="phbw", bufs=3) as phbw, \
                 tc.tile_pool(name="psab", bufs=1, space="PSUM") as psab:
              with tc.tile_pool(name="phax", bufs=2) as phax, \
                 tc.tile_pool(name="phaq", bufs=2) as phaq, \
                 tc.tile_pool(name="phat", bufs=2) as phat:

                KQ = KD // 4        # k-tiles per xt quarter

                def load_xt_quarter(col0, quar, tagname, nspl=1,
                                    eng=None):
                    """One quarter of a chunk's x^T panel; the four quarter
                    tiles are shared by all m-groups (no per-group reloads)
                    and rotate a 5-buffer pool so the next chunk's loads
                    overlap this chunk's last-group reads."""
                    xt_sb = phax.tile([P, KQ, QCH], BF16, tag="xt", bufs=5,
                                      name=tagname)
                    ch = col0 // QCH
                    qk = KQ // nspl
                    for q4 in range(nspl):
                        (eng or nc.sync).dma_start(
                            xt_sb[:, q4 * qk:(q4 + 1) * qk, :],
                            xt_d[:, ch,
                                 quar * KQ + q4 * qk:
                                 quar * KQ + (q4 + 1) * qk, :])
                    return xt_sb

                # interleave the critical first loads across both queues;
                # after startup the scalar queue carries ONLY exp activations
                # (a 600ns DMA-descriptor issue ahead of an exp delays the
                # whole attention pipeline on the in-order scalar queue)
                # cold start: three issue queues run in parallel.  k0 is
                # split across sync+scalar (first matmul gates on it); wq
                # k1-k7 ride the otherwise-idle gpsimd queue (emitted BEFORE
                # the warm-up AllGather, which then blocks that queue for
                # ~70us); xt quarter-0 subloads alternate sync/scalar in
                # consumption order.
                # cold start: three issue queues in parallel.  k0 is split
                # across sync+scalar (the first matmul gates on it); wq
                # k1-k7 ride the otherwise-idle gpsimd queue (software DMA
                # path, ~18GB/s, emitted BEFORE the warm-up AllGather which
                # then blocks that queue); xt quarter-0 subloads alternate
                # sync/scalar in consumption order.
                HFL = FLOC // 2
                nc.scalar.dma_start(wq_sb[:, 0, 0:HFL], wqkv_d[:, 0, 0:HFL])
                nc.sync.dma_start(wq_sb[:, 0, HFL:FLOC],
                                  wqkv_d[:, 0, HFL:FLOC])
                for ko in range(1, 8):
                    nc.gpsimd.dma_start(wq_sb[:, ko, :], wqkv_d[:, ko, :])
                nc.gpsimd.collective_compute(
                    "AllGather", mybir.AluOpType.bypass,
                    replica_groups=[list(range(NCORES))],
                    ins=[warm_in.opt()], outs=[warm_out.opt()])
                xtf0 = phax.tile([P, KQ, QCH], BF16, tag="xt", bufs=5,
                                 name="xtf0")
                for a, b, eng in ((0, 1, nc.sync), (1, 2, nc.scalar),
                                  (2, 4, nc.sync), (4, 6, nc.scalar),
                                  (6, 8, nc.sync)):
                    eng.dma_start(xtf0[:, a:b, :], xt_d[:, 0, a:b, :])
                xt_q0 = [xtf0]
                # rope tables for position range [0,512) (used by chunk 0 of
                # both batches), then the rest
                nc.scalar.dma_start(sc_sb[:, 0:QCH], sc_d[:, 0:QCH])
                nc.scalar.dma_start(sc_sb[:, S:S + QCH], sc_d[:, S:S + QCH])

                # weight feed split across both queues; early tiles go
                # individually (arrival order matches the first chunk's
                # kg-order consumption), later ones as multi-tile range
                # DMAs whose per-partition-contiguous descriptors run at
                # full engine rate
                def wq_span(k0, k1, step=1):
                    for ko in range(k0, k1, step):
                        hi = min(ko + step, k1)
                        eng = nc.sync if (ko // step) % 2 == 0 else nc.scalar
                        eng.dma_start(
                            wq_sb[:, ko:hi, :], wqkv_d[:, ko:hi, :])

                xt_q0.append(load_xt_quarter(0, 1, "xtf1", nspl=2))
                wq_span(8, 15, step=2)
                xt_q0.append(load_xt_quarter(0, 2, "xtf2", nspl=2))
                wq_span(15, 23, step=4)
                xt_q0.append(load_xt_quarter(0, 3, "xtf3"))
                wq_span(23, KD, step=4)
                # diag masks + rest of rope tables
                nc.sync.dma_start(
                    mk[:], mask_d[:].rearrange("(mb p) q -> p mb q", p=P))
                nc.sync.dma_start(sc_sb[:, QCH:S], sc_d[:, QCH:S])
                nc.sync.dma_start(sc_sb[:, S + QCH:2 * S],
                                  sc_d[:, S + QCH:2 * S])

                # k and v ride in the FIRST group: attention's score matmuls
                # need this chunk's roped kT (emitting k's rope first means it
                # completes ~20us before attention reaches the PE queue,
                # instead of being the very last DVE op).  Three groups of 2
                # also mean each group's PSUM allocs reuse buffers freed
                # early in the group-before-last — no alloc-wait micro-stalls
                # (2 live + 2 next = 4 bufs exactly).  Chunk 0 instead uses
                # two groups of 3: a 2-wide group consumes the xt panel at
                # ~240GB/s, outpacing the cold DMA ramp — 3-wide matches it.
                m_groups = [[MT - 2, MT - 1], [0, 1], [2, 3]]
                m_groups0 = [[MT - 2, MT - 1, 0], [1, 2, 3]]

                def rope_store(pss, dst, s0):
                    """dst(bf16) = pss*cos + rot_half(pss)*sin.

                    The multiplies read PSUM f32 (1 elem/cycle on DVE — a
                    partition-shifted tensor_tensor needs one non-SBUF
                    input anyway); t1/t2 are bf16 so the final add runs at
                    2 elem/cycle.  DVE cost ~2.5us per tile.
                    """
                    t1 = phat.tile([P, QCH], BF16, tag="t1",
                                   name=f"t1_{rope_store.n}")
                    t2 = phat.tile([P, QCH], BF16, tag="t2",
                                   name=f"t2_{rope_store.n}")
                    nc.vector.tensor_mul(t1[:], pss[:], cosT[:, s0:s0 + QCH])
                    nc.vector.tensor_mul(t2[0:HHD, :], pss[HHD:P, :],
                                         sinT[0:HHD, s0:s0 + QCH])
                    nc.vector.tensor_mul(t2[HHD:P, :], pss[0:HHD, :],
                                         sinT[HHD:P, s0:s0 + QCH])
                    nc.vector.tensor_add(dst, t1[:], t2[:])
                    rope_store.n += 1
                rope_store.n = 0

                def proj_filler(b, cb, q_dst):
                    """Chunk (b,cb)'s projection as a filler: fill(n) emits
                    up to n matmuls (returns False when exhausted), with the
                    group-boundary DVE work (ropes / v copy / transposes)
                    emitted as the generator crosses each group.  The
                    previous chunk's attention weaves these matmuls between
                    its exp-paced score/PV pairs, so the PE never drains
                    while the scalar engine catches up."""
                    ch = b * NQC + cb
                    col0 = ch * QCH
                    s0 = col0 % S
                    vT = phaq.tile([P, QCH], BF16, tag="vt", name=f"vT{ch}")
                    # quarters 0/1 load eagerly; 2/3 lazily (emitted when
                    # the generator enters the previous quarter).  An
                    # eagerly-emitted quarter DMA carries a WAR wait on the
                    # previous chunk's last reads — with the weave those
                    # now happen much later, and that wait would block the
                    # whole sync queue head-of-line (measured 35us stall).
                    xtq = list(xt_q0) if ch == 0 else [None] * 4

                    def get_q(quar):
                        if xtq[quar] is None:
                            # chunk 1 loads during the cold ramp, while the
                            # PE is still DMA-paced through chunk 0: its
                            # quarter DMAs carry WAR waits on chunk-0 reads
                            # that would head-of-line block the sync queue,
                            # so alternate its quarters onto scalar (idle
                            # there) to decouple the cascade
                            eng = (nc.scalar if (ch == 1 and quar % 2 == 1)
                                   else nc.sync)
                            xtq[quar] = load_xt_quarter(
                                col0, quar, f"xt{ch}_{quar}", eng=eng)
                        return xtq[quar]
                    if ch != 0:
                        get_q(0)
                        get_q(1)

                    def dve_for(grp, pss):
                        # emit DVE consumers in group-list order (k first)
                        for m in grp:
                            if m == MT - 1:          # v
                                nc.vector.tensor_copy(vT[:], pss[m][:])
                                # V^T -> V transposes: sync queue normally
                                # (a scalar-queue DMA_TRANSPOSE costs 1.2us
                                # of queue occupancy right where attention's
                                # exps live).  The DEFERRED chunk's
                                # attention runs much later (phase C), so
                                # its transposes go on the then-idle scalar
                                # queue instead of behind ~4MB of xt loads
                                # on sync (which starved its PV by ~7us).
                                eng_t = (nc.scalar if ch == NB * NQC - 1
                                         else nc.sync)
                                for j in range(KTC):
                                    kt = cb * KTC + j
                                    eng_t.dma_start_transpose(
                                        v_kd[:, b * NKT + kt, :],
                                        vT[:, j * KT:(j + 1) * KT])
                            elif m == MT - 2:        # k
                                rope_store(pss[m], kT[:, col0:col0 + QCH],
                                           s0)
                            else:                    # q
                                rope_store(pss[m], q_dst[:, m, :], s0)

                    def gen():
                        for grp in (m_groups0 if ch == 0 else m_groups):
                            pss = {m: psab.tile([P, QCH], F32, tag="pa",
                                                bufs=4, name=f"pa{ch}_{m}")
                                   for m in grp}
                            for kg in range(KD):
                                quar = kg // KQ
                                xt_sb = get_q(quar)
                                if kg % KQ == 0 and quar < 3:
                                    get_q(quar + 1)
                                for m in grp:
                                    nc.tensor.matmul(
                                        pss[m][:],
                                        wq_sb[:, kg, m * P:(m + 1) * P],
                                        xt_sb[:, kg % KQ, :],
                                        start=(kg == 0),
                                        stop=(kg == KD - 1))
                                    yield True
                            dve_for(grp, pss)

                    g = gen()

                    def fill(n=1):
                        for _ in range(n):
                            if next(g, None) is None:
                                return False
                        return True
                    return fill

                def attn_chunk(ci, b, qc, q_t, full_exp, fill=None):
                    kts = list(range(KTC * (qc + 1)))   # causal prefill
                    # d_ps lives in the "st" pool (scores then run with a
                    # 2-deep lookahead — enough, because fill() weaves proj
                    # matmuls between each score and its exp-gated PV)
                    d_ps = psab.tile([HPC, QCH], F32, tag="st", bufs=3,
                                     name=f"den{ci}")
                    o_tiles = {}
                    n_dgrp = (len(kts) + DGRP - 1) // DGRP
                    dtot = n_dgrp * HPC
                    dcnt = 0
                    step = 0
                    pending = []      # (step_ready, head, dacc)

                    def emit_denoms(lag):
                        nonlocal dcnt
                        while pending and step - pending[0][0] >= lag:
                            _, dh, dacc = pending.pop(0)
                            nc.tensor.matmul(
                                d_ps[:], ecol[:, dh, :], dacc[:],
                                start=(dcnt == 0), stop=(dcnt == dtot - 1))
                            dcnt += 1

                    for h in range(HPC):
                        o_ps = psab.tile([P, QCH], F32, tag="outT", bufs=1,
                                         name=f"o{ci}_{h}")
                        dacc = None
                        dacc_n = 0
                        for i, kt in enumerate(kts):
                            emit_denoms(lag=3)
                            j = kt - KTC * qc
                            qlo = 128 * j if (j >= 1 and not full_exp) else 0
                            st = psab.tile([P, QCH], F32, tag="st", bufs=3,
                                           name=f"st{ci}_{h}_{i}")
                            nc.tensor.matmul(
                                st[:, qlo:QCH],
                                kT[:, b * S + kt * KT:b * S + (kt + 1) * KT],
                                q_t[:, h, qlo:QCH],
                                start=True, stop=True)
                            pt = phbw.tile([P, QCH], BF16, tag="pt", bufs=7,
                                           name=f"pt{ci}_{h}_{i}")
                            nc.scalar.activation(
                                pt[:, qlo:QCH], st[:, qlo:QCH],
                                mybir.ActivationFunctionType.Exp,
                                bias=0.0, scale=float(SCALE))
                            if j >= 0:   # diagonal: mask + zero stale prefix
                                mhi = min(128 * (j + 1), QCH)
                                nc.vector.tensor_mul(pt[:, 0:mhi],
                                                     pt[:, 0:mhi],
                                                     mk[:, j, 0:mhi])
                            if fill is not None:
                                # weave proj matmuls of the NEXT chunk here:
                                # the PE covers this block's exp (+mask)
                                # latency with useful work instead of
                                # idling in-order behind the PV
                                fill(3 if j >= 0 else 2)
                            first, last = (i == 0), (i == len(kts) - 1)
                            nc.tensor.matmul(
                                o_ps[:, qlo:QCH],
                                v_kd[:, b * NKT + kt, :], pt[:, qlo:QCH],
                                start=first, stop=last)
                            # batch up to DGRP exp blocks per denom matmul;
                            # diagonal blocks only contribute on [qlo:] (the
                            # prefix is masked to zero), so restrict the add
                            gpos = i % DGRP
                            if gpos == 0:
                                dacc, dacc_n = pt, 1
                            else:
                                if dacc_n == 1:
                                    dsum = phbw.tile([P, QCH], BF16,
                                                     tag="dsum", bufs=2,
                                                     name=f"ds{ci}_{h}_{i}")
                                    nc.vector.tensor_add(dsum[:], dacc[:],
                                                         pt[:])
                                    dacc = dsum
                                else:
                                    nc.vector.tensor_add(dacc[:, qlo:QCH],
                                                         dacc[:, qlo:QCH],
                                                         pt[:, qlo:QCH])
                                dacc_n += 1
                            if gpos == DGRP - 1 or last:
                                pending.append((step, h, dacc))
                            step += 1
                        o_sb = phbw.tile([P, QCH], BF16, tag="osbuf", bufs=5,
                                         name=f"ou{ci}_{h}")
                        nc.vector.tensor_copy(o_sb[:], o_ps[:])
                        o_tiles[h] = o_sb
                        if fill is not None:
                            fill(2)
                    # flush any remaining denominator matmuls
                    step += 1000
                    emit_denoms(lag=0)
                    assert dcnt == dtot, (dcnt, dtot)
                    inv = phbw.tile([HPC, QCH], F32, tag="inv", bufs=1,
                                    name=f"inv{ci}")
                    nc.vector.reciprocal(inv[:], d_ps[:])
                    invb = phbw.tile([HPC, QCH], BF16, tag="invb", bufs=1,
                                     name=f"invb{ci}")
                    nc.vector.tensor_copy(invb[:], inv[:])
                    # flatten the 4 inv rows onto partition 0 (DMA crosses
                    # partitions freely) so partition_broadcast can expand
                    # them; on the scalar queue, where a short wait on invb
                    # blocks nothing (next exps are ~15us away)
                    invf = phbw.tile([1, HPC * QCH], BF16, tag="invf",
                                     bufs=1, name=f"invf{ci}")
                    nc.scalar.dma_start(invf[:], invb[:])

                    def finalize():
                        # normalization tail; emitted mid-way through the
                        # NEXT projection chunk (before its ropes join the
                        # vector queue).  The inv broadcast runs on GpSimd
                        # (otherwise idle) instead of PE matmuls.
                        for h in range(HPC):
                            bcc = phbw.tile([P, QCH], BF16, tag="bcc",
                                            bufs=1, name=f"bcc{ci}_{h}")
                            nc.gpsimd.partition_broadcast(
                                bcc[:], invf[:, h * QCH:(h + 1) * QCH])
                            at = phbw.tile([P, QCH], BF16, tag="at", bufs=1,
                                           name=f"at{ci}_{h}")
                            nc.vector.tensor_mul(at[:], o_tiles[h][:],
                                                 bcc[:])
                            nc.sync.dma_start(
                                bounce[ci][h * P:(h + 1) * P, :], at[:])
                        nc.gpsimd.collective_compute(
                            "AllGather", mybir.AluOpType.bypass,
                            replica_groups=[list(range(NCORES))],
                            ins=[bounce[ci].opt()], outs=[agc[ci].opt()])
                    return finalize

                # full-prefill causal schedule: attention for chunk qc is
                # WOVEN into the next chunk's projection (its fill matmuls
                # hide the exp chain); the very last attention chunk is
                # deferred past the proj-pool close so phase C's first
                # matmuls can fill its exp-latency stalls
                deferred = (NB - 1, NQC - 1)
                wo_sched = {1: (0, 5), 2: (5, 10), 3: (10, 14),
            4: (14, 19), 5: (19, 23), 6: (23, 28),
            7: (28, KD)}
                chunk_no = 0
                prev = None       # previous chunk's attention, not yet run
                for b in range(NB):
                    for cb in range(NQC):
                        if (b, cb) == deferred:
                            q_dst = q_def
                        else:
                            q_dst = phaq.tile([P, HPC, QCH], BF16, tag="qch",
                                              name=f"q{b}_{cb}")
                        fill = proj_filler(b, cb, q_dst)
                        if prev is not None:
                            fin = attn_chunk(*prev, fill=fill)
                            # normalization + AllGather for the chunk that
                            # just finished its attention
                            fin()
                        while fill(16):
                            pass
                        # trickle wo weight loads behind the xt streams
                        chunk_no += 1
                        if chunk_no in wo_sched:
                            lo, hi = wo_sched[chunk_no]
                            nc.sync.dma_start(wo_sb[:, lo:hi, :],
                                              wo_d[:, lo:hi, :])
                        if chunk_no == 4:
                            # prefetch phase-C chunk 0's first panel quarter
                            # into its persistent home (AG0 finished long
                            # ago; both queues have slack mid-phase)
                            nc.sync.dma_start(
                                agt0q[:],
                                agc[0][0:NPQ * P, :]
                                .rearrange("(ko p) t -> p ko t", p=P))
                        if (b, cb) != deferred:
                            ci = b * NQC + cb
                            prev = (ci, b, cb, q_dst,
                                    b == 0 and cb == 0)
                        else:
                            prev = None

              # -------------- Phase C: out^T = wo_c^T @ attn^T ---------------
              with tc.tile_pool(name="phcx", bufs=2) as phcx, \
                 tc.tile_pool(name="phco", bufs=2) as phco:
                def load_agt(ci, skip_q0=False):
                    agt = phcx.tile([P, KD, QCH], BF16, tag="agt",
                                    name=f"agt{ci}")
                    kq = KD // 4
                    for quar in range(4):
                        # first quarter split across both queues so the
                        # first matmuls start as early as possible
                        if quar == 0:
                            if skip_q0:
                                # k-tiles [0:NPQ) served by the persistent
                                # agt0q; load only the remainder
                                nc.sync.dma_start(
                                    agt[:, NPQ:kq, :],
                                    agc[ci][NPQ * P:kq * P, :]
                                    .rearrange("(ko p) t -> p ko t", p=P))
                                continue
                            h = kq // 2
                            for sub, eng in ((0, nc.sync), (1, nc.scalar)):
                                eng.dma_start(
                                    agt[:, sub * h:(sub + 1) * h, :],
                                    agc[ci][sub * h * P:(sub + 1) * h * P, :]
                                    .rearrange("(ko p) t -> p ko t", p=P))
                            continue
                        eng = nc.scalar if quar % 2 == 1 else nc.sync
                        eng.dma_start(
                            agt[:, quar * kq:(quar + 1) * kq, :],
                            agc[ci][quar * kq * P:(quar + 1) * kq * P, :]
                            .rearrange("(ko p) t -> p ko t", p=P))
                    return agt

                # chunk 0's panel loads are issued BEFORE the deferred
                # attention floods the scalar queue with exps; otherwise
                # phase C's first matmuls wait ~15us for the scalar-queue
                # half of the panel
                agt0 = load_agt(0, skip_q0=True)
                def_fin = attn_chunk(deferred[0] * NQC + deferred[1],
                                     deferred[0], deferred[1], q_def, False)
                NMD = ODPC // P
                for ci in range(NCHK):
                    tok0 = ci * QCH
                    agt = agt0 if ci == 0 else load_agt(ci)
                    kq = KD // 4
                    # quarter-major accumulation: the first 4*kq matmuls
                    # depend only on the first agt quarter, so phase C
                    # pipelines against the quarter DMAs instead of
                    # waiting for the whole 4MB panel
                    pos = [psab.tile([P, QCH], F32, tag="pa", bufs=4,
                                     name=f"po{ci}_{md}")
                           for md in range(NMD)]
                    for quar in range(4):
                        for md in range(NMD):
                            for kf in range(quar * kq, (quar + 1) * kq):
                                src = (agt0q if (ci == 0 and kf < NPQ)
                                       else agt)
                                nc.tensor.matmul(
                                    pos[md][:],
                                    wo_sb[:, kf, md * P:(md + 1) * P],
                                    src[:, kf, :],
                                    start=(kf == 0), stop=(kf == KD - 1))
                            if quar == 3:
                                # drain each bank as soon as its group stops
                                osb = phco.tile([P, QCH], BF16, tag="osb",
                                                bufs=2, name=f"osb{ci}_{md}")
                                nc.vector.tensor_copy(osb[:], pos[md][:])
                                eng = nc.scalar if (ci >= 1 and md % 2 == 1) \
                                    else nc.sync
                                eng.dma_start(
                                    out_d[md * P:(md + 1) * P,
                                          tok0:tok0 + QCH], osb[:])
                    if ci == 0 and def_fin is not None:
                        def_fin()
                        def_fin = None

    nc.compile()
    return nc


def _host_prep(x, wqkv, wo, sincos, full_causal_mask, start_pos,
               NB, S, D, HPC, NCORES):
    """Shard, cast, and lay out inputs; verify the causal-mask structure."""
    bf16 = ml_dtypes.bfloat16
    TOK = NB * S
    H = HPC * NCORES
    QF = HPC * HD
    NQC = S // QCH
    NKT = S // KT
    KTC = QCH // KT
    ODPC = D // NCORES
    q_sz = H * HD

    # partition-major xt: [p, chunk, ko, t] — per-partition contiguous runs
    # of KD*QCH*2 bytes per chunk so xt DMAs use multi-KB descriptors
    NCHK = TOK // QCH
    KD = D // P
    xt = np.ascontiguousarray(
        x.reshape(NCHK, QCH, KD, P).transpose(3, 0, 2, 1)).astype(bf16)

    # effective mask: [q, k] (batch-shared), incl. the cache-validity term
    m_eff = np.asarray(full_causal_mask[0, 0], dtype=bool)
    m_eff = m_eff[start_pos:start_pos + S, :S].copy()
    valid = np.arange(S) < (start_pos + S)
    m_eff &= valid[None, :]

    # the kernel hardcodes a block-causal structure: block (qc, kt) is full
    # for kt < KTC*qc, diagonal-j (canonical pattern) for kt = KTC*qc + j,
    # empty above.  Verify the provided mask matches.
    qi = np.arange(QCH)
    for qc in range(NQC):
        for kt in range(NKT):
            blk = m_eff[qc * QCH:(qc + 1) * QCH, kt * KT:(kt + 1) * KT]
            j = kt - KTC * qc
            if j < 0:
                assert blk.all(), f"block ({qc},{kt}) expected full"
            elif j < KTC:
                exp_blk = (qi[:, None] >= 128 * j + np.arange(KT)[None, :])
                assert np.array_equal(blk, exp_blk), \
                    f"block ({qc},{kt}) unexpected diagonal pattern"
            else:
                assert not blk.any(), f"block ({qc},{kt}) expected empty"

    # canonical diagonal masks, [k, q] layout, one per j: cols [0,128j) = 0
    # (zeroes the stale prefix), cols [128j, 512): 1 where q >= 128j + k
    maskblk = np.zeros((KTC, KT, QCH), dtype=np.float32)
    for j in range(KTC):
        maskblk[j] = (qi[None, :] >= 128 * j + np.arange(KT)[:, None])
        maskblk[j, :, :128 * j] = 0.0
    maskblk = maskblk.reshape(KTC * KT, QCH).astype(bf16)

    # rope tables, transposed + duplicated halves; sin rows 0:64 negated
    sc = np.asarray(sincos[start_pos:start_pos + S], dtype=np.float32)
    sin, cos = sc[:, :HHD], sc[:, HHD:]
    cosT2 = np.concatenate([cos.T, cos.T], axis=0)           # [128, S]
    sinT2 = np.concatenate([-sin.T, sin.T], axis=0)          # [128, S]
    sincos2 = np.concatenate([cosT2, sinT2], axis=1).astype(bf16)

    in_maps = []
    for c in range(NCORES):
        qcols = np.asarray(wqkv[:, c * QF:(c + 1) * QF])
        kcols = np.asarray(wqkv[:, q_sz + c * HD:q_sz + (c + 1) * HD])
        vcols = np.asarray(
            wqkv[:, q_sz + NCORES * HD + c * HD:
                 q_sz + NCORES * HD + (c + 1) * HD])
        wqkv_c = np.concatenate([qcols, kcols, vcols], axis=1)
        # partition-major weights: [p, ko, cols]
        wqkv_c = np.ascontiguousarray(
            wqkv_c.reshape(KD, P, -1).transpose(1, 0, 2)).astype(bf16)
        wo_c = np.ascontiguousarray(
            np.asarray(wo[:, c * ODPC:(c + 1) * ODPC])
            .reshape(KD, P, ODPC).transpose(1, 0, 2)).astype(bf16)
        in_maps.append({
            "xt": xt, "wqkv": wqkv_c, "wo": wo_c,
            "sincos2": sincos2, "maskblk": maskblk,
        })
    return in_maps


_CACHE = {}


def run_distributed(x, wqkv, wo, sincos, full_causal_mask, start_pos,
                    NB, S, D, HPC, NCORES, trace=False, tmpdir=None):
    in_maps = _host_prep(
        x, wqkv, wo, sincos, full_causal_mask, start_pos,
        NB, S, D, HPC, NCORES)
    key = (NB, S, D, HPC, NCORES)
    if key not in _CACHE:
        _CACHE[key] = build_graph(NB, S, D, HPC, NCORES)
    nc = _CACHE[key]
    res = run_bass_kernel_spmd(nc, in_maps, list(range(NCORES)), trace=trace,
                               tmpdir=tmpdir)
    TOK = NB * S
    out = np.empty((TOK, D), dtype=np.float32)
    ODPC = D // NCORES
    for c in range(NCORES):
        out[:, c * ODPC:(c + 1) * ODPC] = \
            np.asarray(res.results[c]["out"], dtype=np.float32).T
    return out.reshape(NB, S, D), res


def kernel(x, wqkv, wo, sincos, cache_k, cache_v, full_causal_mask,
           start_pos) -> np.ndarray:
    x = np.asarray(x)
    start_pos = int(np.asarray(start_pos))
    B, S_, D_ = x.shape
    assert start_pos == 0, "prefill-only kernel (seq fills the whole cache)"
    out, _ = run_distributed(
        x, np.asarray(wqkv), np.asarray(wo), np.asarray(sincos),
        np.asarray(full_causal_mask), start_pos,
        NB=B, S=S_, D=D_, HPC=4, NCORES=8)
    return out

